# revision 8
# baseline (speedup 1.0000x reference)
"""Trainium2 Bass kernel for nn_AttnFreeLayer (linear-attention-style layer).

Computes, for inputs q,k,v [B,S,D] and weights Wq,Wk,Wv [E,D] (E=D):
    q_in = elu(q @ Wq^T) + 1
    k_in = elu(k @ Wk^T) + 1
    v_in = v @ Wv^T
    kv_in = k_in * v_in
    out = q_in * (kv_in + cumsum_s(kv_in)) / cumsum_s(k_in)

Sharding: 8 cores = 4 batches x 2 halves of the output dim E; no
collectives. Each core computes out[b, e0:e0+512, :] in a TRANSPOSED
[e, s] layout: the projection matmuls put W chunks stationary and x^T
moving, so outputs land with e on partitions and s on the free dim.
The seq-cumsum runs along the free dimension via the DVE
tensor_tensor_scan instruction (chained across s-chunks through its
`initial` operand).

v2 restructure vs the original baseline:
- DMA coalescing: q/k/v packed into ONE DRAM tensor laid out so each
  load is a single 24KB-per-partition contiguous transfer (8 x-loads
  for the whole pass instead of 45), and output is staged in SBUF and
  stored 2048 columns at a time (16 stores instead of 64). 27 DMAs
  total vs 118.
- elu(x)+1 = min(exp(x), relu(x)+1)  (e^x >= 1+x everywhere, equality
  only at 0, so the min selects exp(x) for x<0 and x+1 for x>0).
  That needs only ONE table-lookup pass per element on ACT (exp)
  plus a relu pass, vs the 3-pass exp(-relu(-x))+relu(x) chain.
  The +1 runs on DVE tensor_scalar (4x rate), min on tensor_tensor
  (2x rate).
- den_mode="lnexp": 1/k_prefix = exp(-ln(k_prefix)) on ACT (exp, ln,
  relu and copy all live in the SAME activation table set, so no
  table-swap penalty), batched [P, 4*SC] per s-chunk.  Moves the
  1x-rate DVE reciprocal onto the ACT engine. den_mode="recip" keeps
  the DVE reciprocal.

Precision: hybrid fp16/fp8 as before. First 512 seq positions use
fp16 matmuls, the remaining 7680 use fp8-e4m3 DoubleRow matmuls.
fp8 weights pre-scaled by 16 (avoids e4m3 subnormals); downstream ACT
ops undo it via their free `scale`, the DVE relu+1 path folds it into
the relu tensor_scalar.
"""

import sys

for _p in ("/opt/trn_rl_repo",):
    if _p not in sys.path:
        sys.path.insert(0, _p)

from contextlib import ExitStack

import numpy as np
import ml_dtypes

import concourse.bass as bass
import concourse.tile as tile
from concourse import bacc
from concourse import mybir
from concourse.alu_op_type import AluOpType
from concourse.bass_utils import run_bass_kernel_spmd

FP8 = mybir.dt.float8e4
FP16 = mybir.dt.float16
FP32 = mybir.dt.float32
AF = mybir.ActivationFunctionType
DR = mybir.MatmulPerfMode.DoubleRow

B, S, D, E = 4, 8192, 1024, 1024
NCORES = 8
EH = E // 2  # e-half per core
P = 128  # partition block
SC = 512  # s-chunk width (PSUM bank = 512 fp32)
N_SC = S // SC  # 16
N_EC = EH // P  # 4
ND = D // P  # 8 contraction chunks
WS = 16.0  # fp8 weight prescale
SCL = 1024  # x-load width (2 s-chunks per DMA)
NL = S // SCL  # 8 loads
SCO = 2048  # out-store width (4 s-chunks per DMA)


def build_nc(repeat=1, debug=False, den_mode="recip"):
    """den_mode: "lnexp" (1/ck = exp(-ln(ck)) on ACT) or "recip" (DVE)."""
    nc = bacc.Bacc("TRN2", target_bir_lowering=False, debug=debug)

    # packed inputs: per-partition contiguous 24KB loads
    xa8 = nc.declare_dram_parameter("xa8", [NL, P, 3 * ND * SCL], FP8, isOutput=False)
    xb16 = nc.declare_dram_parameter("xb16", [P, 3 * ND * SC], FP16, isOutput=False)
    wb16 = nc.declare_dram_parameter("wb16", [P, 3 * ND * EH], FP16, isOutput=False)
    wb8 = nc.declare_dram_parameter("wb8", [P, 3 * ND * EH], FP8, isOutput=False)
    outp = nc.declare_dram_parameter("out", [EH, S], FP16, isOutput=True)

    with tile.TileContext(nc) as tc, ExitStack() as ctx:
        wpool = ctx.enter_context(tc.tile_pool(name="w", bufs=1))
        x8pool = ctx.enter_context(tc.tile_pool(name="x8", bufs=2))
        apool = ctx.enter_context(tc.tile_pool(name="act", bufs=2))
        vpool = ctx.enter_context(tc.tile_pool(name="vv", bufs=2))
        kvpool = ctx.enter_context(tc.tile_pool(name="kv", bufs=3))
        cpool = ctx.enter_context(tc.tile_pool(name="cum", bufs=2))
        dpool = ctx.enter_context(tc.tile_pool(name="den", bufs=2))
        opool = ctx.enter_context(tc.tile_pool(name="out", bufs=2))
        pp = ctx.enter_context(tc.tile_pool(name="pqk", bufs=3, space="PSUM"))
        pvp = ctx.enter_context(tc.tile_pool(name="pv", bufs=2, space="PSUM"))

        # --- resident weights + first-chunk fp16 x ---
        w16_t = wpool.tile([P, 3, ND, EH], FP16, tag="w16")
        nc.sync.dma_start(
            out=w16_t[:], in_=wb16[:].rearrange("p (i j e) -> p i j e", i=3, j=ND)
        )
        w8_t = wpool.tile([P, 3, ND, EH], FP8, tag="w8")
        nc.sync.dma_start(
            out=w8_t[:], in_=wb8[:].rearrange("p (i j e) -> p i j e", i=3, j=ND)
        )
        x16_t = wpool.tile([P, 3, ND, SC], FP16, tag="x16")
        nc.sync.dma_start(
            out=x16_t[:], in_=xb16[:].rearrange("p (i j s) -> p i j s", i=3, j=ND)
        )

        def main_body():
            carry_k = [None]  # [P, 4*SC] group tile of prev sc (or None)
            carry_kv = [None]
            otiles = [None] * N_EC

            def emit_unit(sc, ec, x8t, ckg, ckvg):
                fp8 = sc > 0
                e0 = ec * P
                sscale = (1.0 / WS) if fp8 else 1.0
                half = (sc % 2) if fp8 else 0
                pqk = pp.tile([P, 2 * SC], FP32, tag="pqk")
                pv_ = pvp.tile([P, SC], FP32, tag="pv")
                for i in range(3):
                    dst = pqk[:, i * SC : (i + 1) * SC] if i < 2 else pv_[:]
                    if fp8:
                        for j in range(ND // 2):
                            nc.tensor.matmul(
                                dst,
                                lhsT=w8_t[:, i, 2 * j : 2 * j + 2, e0 : e0 + P],
                                rhs=x8t[
                                    :,
                                    i,
                                    2 * j : 2 * j + 2,
                                    half * SC : (half + 1) * SC,
                                ],
                                start=(j == 0),
                                stop=(j == ND // 2 - 1),
                                perf_mode=DR,
                                skip_group_check=True,
                            )
                    else:
                        for j in range(ND):
                            nc.tensor.matmul(
                                dst,
                                lhsT=w16_t[:, i, j, e0 : e0 + P],
                                rhs=x16_t[:, i, j, :],
                                start=(j == 0),
                                stop=(j == ND - 1),
                                skip_group_check=True,
                            )
                # elu(x)+1 = min(exp(x), relu(x)+1)
                ex = apool.tile([P, 2 * SC], FP16, tag="ex")
                nc.scalar.activation(ex[:], pqk[:], AF.Exp, scale=sscale)
                rp = apool.tile([P, 2 * SC], FP16, tag="rp")
                nc.scalar.activation(rp[:], pqk[:], AF.Relu, scale=sscale)
                v1 = vpool.tile([P, SC], FP16, tag="v1")
                nc.scalar.activation(v1[:], pv_[:], AF.Copy, scale=sscale)
                rp1 = apool.tile([P, 2 * SC], FP16, tag="rp1")
                nc.vector.tensor_scalar_add(rp1[:], rp[:], 1.0)
                qk1 = apool.tile([P, 2 * SC], FP16, tag="qk1")
                nc.vector.tensor_tensor(qk1[:], ex[:], rp1[:], AluOpType.min)
                q1 = qk1[:, 0:SC]
                k1 = qk1[:, SC : 2 * SC]
                kv = kvpool.tile([P, SC], FP16, tag="kv")
                nc.vector.tensor_mul(kv[:], k1, v1[:])
                # inclusive cumsums along s, chained across s-chunks
                cs = slice(ec * SC, (ec + 1) * SC)
                ik = 0.0 if sc == 0 else carry_k[0][:, (ec + 1) * SC - 1 : (ec + 1) * SC]
                ikv = (
                    0.0 if sc == 0 else carry_kv[0][:, (ec + 1) * SC - 1 : (ec + 1) * SC]
                )
                nc.vector.tensor_tensor_scan(
                    ckg[:, cs], k1, k1, ik, op0=AluOpType.add, op1=AluOpType.bypass
                )
                nc.vector.tensor_tensor_scan(
                    ckvg[:, cs], kv[:], kv[:], ikv, op0=AluOpType.add, op1=AluOpType.bypass
                )
                # num = kv + ckv; t1 = q1 * num
                num = kvpool.tile([P, SC], FP16, tag="num")
                nc.vector.tensor_add(num[:], ckvg[:, cs], kv[:])
                t1 = kvpool.tile([P, SC], FP16, tag="t1")
                nc.vector.tensor_mul(t1[:], q1, num[:])
                return t1

            x8t = None
            for sc in range(N_SC):
                # fp8 chunk sc lives in load l=sc//2, half sc%2; a new load
                # is needed at sc=1 (l=0) and every even sc>=2 (l advances)
                if sc == 1 or (sc >= 2 and sc % 2 == 0):
                    x8t = x8pool.tile([P, 3, ND, SCL], FP8, tag="x8")
                    nc.sync.dma_start(
                        out=x8t[:],
                        in_=xa8[sc // 2, :, :].rearrange(
                            "p (i j s) -> p i j s", i=3, j=ND
                        ),
                    )
                g = sc % (SCO // SC)  # position within the out-store group
                if g == 0:
                    for ec in range(N_EC):
                        otiles[ec] = opool.tile(
                            [P, SCO], FP16, tag=f"ot{ec}", name=f"ot{ec}"
                        )
                ckg = cpool.tile([P, N_EC * SC], FP16, tag="ckg")
                ckvg = cpool.tile([P, N_EC * SC], FP16, tag="ckvg")
                deng = dpool.tile([P, N_EC * SC], FP16, tag="deng")
                t1s = [emit_unit(sc, ec, x8t, ckg, ckvg) for ec in range(N_EC)]
                # 1/ck for all 4 ec blocks of this s-chunk
                if den_mode == "lnexp":
                    lnk = dpool.tile([P, N_EC * SC], FP16, tag="lnk")
                    nc.scalar.activation(lnk[:], ckg[:], AF.Ln)
                    nc.scalar.activation(deng[:], lnk[:], AF.Exp, scale=-1.0)
                else:
                    with nc.allow_low_precision(reason="1/k_prefix; 5e-4 rel ok"):
                        nc.vector.reciprocal(deng[:], ckg[:])
                for ec in range(N_EC):
                    cs = slice(ec * SC, (ec + 1) * SC)
                    nc.vector.tensor_mul(
                        otiles[ec][:, g * SC : (g + 1) * SC], t1s[ec], deng[:, cs]
                    )
                if g == SCO // SC - 1:
                    for ec in range(N_EC):
                        nc.sync.dma_start(
                            out=outp[
                                ec * P : (ec + 1) * P,
                                (sc - g) * SC : (sc + 1) * SC,
                            ],
                            in_=otiles[ec][:],
                        )
                carry_k[0], carry_kv[0] = ckg, ckvg

        if repeat == 1:
            main_body()
        else:
            with tc.For_i(0, repeat, 1):
                main_body()

    nc.compile()
    return nc


def _e4m3(x):
    return np.clip(x, -240, 240).astype(ml_dtypes.float8_e4m3)


def _host_prep(v, k, q, Wq, Wk, Wv):
    """Build the 8 per-core input maps (packed, per-partition contiguous)."""
    xa8_b, xb16_b = {}, {}
    for b in range(B):
        a8 = np.empty((NL, P, 3, ND, SCL), dtype=ml_dtypes.float8_e4m3)
        b16 = np.empty((P, 3, ND, SC), dtype=np.float16)
        for i, x in enumerate((q, k, v)):
            # x[b]: [S, D] -> chunks [NL, SCL, ND, P] -> [NL, P, ND, SCL]
            x8 = _e4m3(np.asarray(x[b], np.float32))
            a8[:, :, i] = x8.reshape(NL, SCL, ND, P).transpose(0, 3, 2, 1)
            b16[:, i] = (
                np.asarray(x[b][:SC], np.float32)
                .astype(np.float16)
                .reshape(SC, ND, P)
                .transpose(2, 1, 0)
            )
        xa8_b[b] = a8.reshape(NL, P, 3 * ND * SCL)
        xb16_b[b] = b16.reshape(P, 3 * ND * SC)
    in_maps = []
    for c in range(NCORES):
        b, h = c // 2, c % 2
        e0 = h * EH
        w16 = np.empty((P, 3, ND, EH), dtype=np.float16)
        w8 = np.empty((P, 3, ND, EH), dtype=ml_dtypes.float8_e4m3)
        for i, W in enumerate((Wq, Wk, Wv)):
            # W^T[:, e0:e0+EH]: [D, EH] -> [ND, P, EH] -> [P, ND, EH]
            wt = np.ascontiguousarray(np.asarray(W, np.float32).T[:, e0 : e0 + EH])
            w16[:, i] = wt.astype(np.float16).reshape(ND, P, EH).transpose(1, 0, 2)
            w8[:, i] = _e4m3(wt * WS).reshape(ND, P, EH).transpose(1, 0, 2)
        in_maps.append(
            {
                "xa8": xa8_b[b],
                "xb16": xb16_b[b],
                "wb16": w16.reshape(P, 3 * ND * EH),
                "wb8": w8.reshape(P, 3 * ND * EH),
            }
        )
    return in_maps


_NC_CACHE = None


def _get_nc():
    global _NC_CACHE
    if _NC_CACHE is None:
        _NC_CACHE = build_nc()
    return _NC_CACHE


def run_spmd(v, k, q, Wq, Wk, Wv, **kwargs):
    """Run on 8 cores; returns (assembled output [B,S,E] fp32, raw results)."""
    nc = _get_nc()
    in_maps = _host_prep(v, k, q, Wq, Wk, Wv)
    res = run_bass_kernel_spmd(nc, in_maps, core_ids=list(range(NCORES)), **kwargs)
    full = np.empty((B, S, E), dtype=np.float32)
    for c in range(NCORES):
        b, h = c // 2, c % 2
        full[b, :, h * EH : (h + 1) * EH] = res.results[c]["out"].T.astype(np.float32)
    return full, res


def kernel(v, k, q, Wq, Wk, Wv):
    v, k, q, Wq, Wk, Wv = (
        np.asarray(a, dtype=np.float32) for a in (v, k, q, Wq, Wk, Wv)
    )
    full, _ = run_spmd(v, k, q, Wq, Wk, Wv)
    return full


# revision 14
# speedup vs baseline: 8.1640x; 8.1640x over previous
"""Trainium2 Bass kernel for nn_AttnFreeLayer (linear-attention-style layer).

Computes, for inputs q,k,v [B,S,D] and weights Wq,Wk,Wv [E,D] (E=D):
    q_in = elu(q @ Wq^T) + 1
    k_in = elu(k @ Wk^T) + 1
    v_in = v @ Wv^T
    kv_in = k_in * v_in
    out = q_in * (kv_in + cumsum_s(kv_in)) / cumsum_s(k_in)

Sharding: 8 cores = 4 batches x 2 halves of the output dim E; no
collectives. Each core computes out[b, e0:e0+512, :] in a TRANSPOSED
[e, s] layout: the projection matmuls put W chunks stationary and x^T
moving, so outputs land with e on partitions and s on the free dim.
The seq-cumsum runs along the free dimension via the DVE
tensor_tensor_scan instruction (chained across s-chunks through its
`initial` operand).

v2 restructure vs the original baseline:
- DMA coalescing: q/k/v packed into ONE DRAM tensor laid out so each
  load is a single 24KB-per-partition contiguous transfer (8 x-loads
  for the whole pass instead of 45), and output is staged in SBUF and
  stored 2048 columns at a time (16 stores instead of 64). 27 DMAs
  total vs 118.
- elu(x)+1 = min(exp(x), relu(x)+1)  (e^x >= 1+x everywhere, equality
  only at 0, so the min selects exp(x) for x<0 and x+1 for x>0).
  That needs only ONE table-lookup pass per element on ACT (exp)
  plus a relu pass, vs the 3-pass exp(-relu(-x))+relu(x) chain.
  The +1 runs on DVE tensor_scalar (4x rate), min on tensor_tensor
  (2x rate).
- den_mode="lnexp": 1/k_prefix = exp(-ln(k_prefix)) on ACT (exp, ln,
  relu and copy all live in the SAME activation table set, so no
  table-swap penalty), batched [P, 4*SC] per s-chunk.  Moves the
  1x-rate DVE reciprocal onto the ACT engine. den_mode="recip" keeps
  the DVE reciprocal.

Precision: hybrid fp16/fp8 as before. First 512 seq positions use
fp16 matmuls, the remaining 7680 use fp8-e4m3 DoubleRow matmuls.
fp8 weights pre-scaled by 16 (avoids e4m3 subnormals); downstream ACT
ops undo it via their free `scale`, the DVE relu+1 path folds it into
the relu tensor_scalar.
"""

import sys

for _p in ("/opt/trn_rl_repo",):
    if _p not in sys.path:
        sys.path.insert(0, _p)

from contextlib import ExitStack

import numpy as np
import ml_dtypes

import concourse.bass as bass
import concourse.tile as tile
from concourse import bacc
from concourse import mybir
from concourse.alu_op_type import AluOpType
from concourse.bass_utils import run_bass_kernel_spmd

FP8 = mybir.dt.float8e4
FP16 = mybir.dt.float16
FP32 = mybir.dt.float32
AF = mybir.ActivationFunctionType
DR = mybir.MatmulPerfMode.DoubleRow

B, S, D, E = 4, 8192, 1024, 1024
NCORES = 8
EH = E // 2  # e-half per core
P = 128  # partition block
SC = 512  # s-chunk width (PSUM bank = 512 fp32)
N_SC = S // SC  # 16
N_EC = EH // P  # 4
ND = D // P  # 8 contraction chunks
WS = 16.0  # fp8 weight prescale
SCL = 1024  # x-load width (2 s-chunks per DMA)
NL = S // SCL  # 8 loads
SCO = 2048  # out-store width (4 s-chunks per DMA)


def build_nc(repeat=1, debug=False, den_mode="recip", scan_eng="vector", gps=False):
    """den_mode: "lnexp" (1/ck = exp(-ln(ck)) on ACT) or "recip" (DVE).
    scan_eng: must be "vector" — TensorTensorScanArith is NOT a valid
    opcode on the Pool engine (walrus codegen asserts), so the scans stay
    on DVE at 1x rate (no dve perf mode for scans).
    gps: offload kv-mul and num-add to the GPSIMD/Pool engine (eff 0.42,
    so ~1.1us/unit there vs 0.7us on DVE — only pays if DVE is the
    critical engine AND SBUF-port contention doesn't eat the win)."""
    nc = bacc.Bacc("TRN2", target_bir_lowering=False, debug=debug)

    # packed inputs: per-partition contiguous 24KB loads
    xa8 = nc.declare_dram_parameter("xa8", [NL, P, 3 * ND * SCL], FP8, isOutput=False)
    xb16 = nc.declare_dram_parameter("xb16", [P, 3 * ND * SC], FP16, isOutput=False)
    wb16 = nc.declare_dram_parameter("wb16", [P, 3 * ND * EH], FP16, isOutput=False)
    wb8 = nc.declare_dram_parameter("wb8", [P, 3 * ND * EH], FP8, isOutput=False)
    outp = nc.declare_dram_parameter("out", [EH, S], FP16, isOutput=True)

    with tile.TileContext(nc) as tc, ExitStack() as ctx:
        wpool = ctx.enter_context(tc.tile_pool(name="w", bufs=1))
        x8pool = ctx.enter_context(tc.tile_pool(name="x8", bufs=2))
        apool = ctx.enter_context(tc.tile_pool(name="act", bufs=2))
        vpool = ctx.enter_context(tc.tile_pool(name="vv", bufs=2))
        kvpool = ctx.enter_context(tc.tile_pool(name="kv", bufs=3))
        cpool = ctx.enter_context(tc.tile_pool(name="cum", bufs=2))
        dpool = ctx.enter_context(tc.tile_pool(name="den", bufs=2))
        opool = ctx.enter_context(tc.tile_pool(name="out", bufs=2))
        pp = ctx.enter_context(tc.tile_pool(name="pqk", bufs=3, space="PSUM"))
        pvp = ctx.enter_context(tc.tile_pool(name="pv", bufs=2, space="PSUM"))

        # --- resident weights + first-chunk fp16 x ---
        w16_t = wpool.tile([P, 3, ND, EH], FP16, tag="w16")
        nc.sync.dma_start(
            out=w16_t[:], in_=wb16[:].rearrange("p (i j e) -> p i j e", i=3, j=ND)
        )
        w8_t = wpool.tile([P, 3, ND, EH], FP8, tag="w8")
        nc.sync.dma_start(
            out=w8_t[:], in_=wb8[:].rearrange("p (i j e) -> p i j e", i=3, j=ND)
        )
        x16_t = wpool.tile([P, 3, ND, SC], FP16, tag="x16")
        nc.sync.dma_start(
            out=x16_t[:], in_=xb16[:].rearrange("p (i j s) -> p i j s", i=3, j=ND)
        )

        def main_body():
            carry_k = [None]  # [P, 4*SC] group tile of prev sc (or None)
            carry_kv = [None]
            otiles = [None] * N_EC

            def emit_unit(sc, ec, x8t, ckg, ckvg):
                fp8 = sc > 0
                e0 = ec * P
                sscale = (1.0 / WS) if fp8 else 1.0
                half = (sc % 2) if fp8 else 0
                pqk = pp.tile([P, 2 * SC], FP32, tag="pqk")
                pv_ = pvp.tile([P, SC], FP32, tag="pv")
                for i in range(3):
                    dst = pqk[:, i * SC : (i + 1) * SC] if i < 2 else pv_[:]
                    if fp8:
                        for j in range(ND // 2):
                            nc.tensor.matmul(
                                dst,
                                lhsT=w8_t[:, i, 2 * j : 2 * j + 2, e0 : e0 + P],
                                rhs=x8t[
                                    :,
                                    i,
                                    2 * j : 2 * j + 2,
                                    half * SC : (half + 1) * SC,
                                ],
                                start=(j == 0),
                                stop=(j == ND // 2 - 1),
                                perf_mode=DR,
                                skip_group_check=True,
                            )
                    else:
                        for j in range(ND):
                            nc.tensor.matmul(
                                dst,
                                lhsT=w16_t[:, i, j, e0 : e0 + P],
                                rhs=x16_t[:, i, j, :],
                                start=(j == 0),
                                stop=(j == ND - 1),
                                skip_group_check=True,
                            )
                # elu(x)+1 = min(exp(x), relu(x)+1)
                ex = apool.tile([P, 2 * SC], FP16, tag="ex")
                nc.scalar.activation(ex[:], pqk[:], AF.Exp, scale=sscale)
                rp = apool.tile([P, 2 * SC], FP16, tag="rp")
                nc.scalar.activation(rp[:], pqk[:], AF.Relu, scale=sscale)
                v1 = vpool.tile([P, SC], FP16, tag="v1")
                nc.scalar.activation(v1[:], pv_[:], AF.Copy, scale=sscale)
                rp1 = apool.tile([P, 2 * SC], FP16, tag="rp1")
                nc.vector.tensor_scalar_add(rp1[:], rp[:], 1.0)
                qk1 = apool.tile([P, 2 * SC], FP16, tag="qk1")
                nc.vector.tensor_tensor(qk1[:], ex[:], rp1[:], AluOpType.min)
                q1 = qk1[:, 0:SC]
                k1 = qk1[:, SC : 2 * SC]
                kv = kvpool.tile([P, SC], FP16, tag="kv")
                (nc.gpsimd if gps else nc.vector).tensor_mul(kv[:], k1, v1[:])
                # inclusive cumsums along s, chained across s-chunks
                cs = slice(ec * SC, (ec + 1) * SC)
                ik = 0.0 if sc == 0 else carry_k[0][:, (ec + 1) * SC - 1 : (ec + 1) * SC]
                ikv = (
                    0.0 if sc == 0 else carry_kv[0][:, (ec + 1) * SC - 1 : (ec + 1) * SC]
                )
                seng = nc.gpsimd if scan_eng == "gpsimd" else nc.vector
                seng.tensor_tensor_scan(
                    ckg[:, cs], k1, k1, ik, op0=AluOpType.add, op1=AluOpType.bypass
                )
                seng.tensor_tensor_scan(
                    ckvg[:, cs], kv[:], kv[:], ikv, op0=AluOpType.add, op1=AluOpType.bypass
                )
                # num = kv + ckv; t1 = q1 * num
                num = kvpool.tile([P, SC], FP16, tag="num")
                (nc.gpsimd if gps else nc.vector).tensor_add(num[:], ckvg[:, cs], kv[:])
                t1 = kvpool.tile([P, SC], FP16, tag="t1")
                nc.vector.tensor_mul(t1[:], q1, num[:])
                return t1

            x8t = None
            for sc in range(N_SC):
                # fp8 chunk sc lives in load l=sc//2, half sc%2; a new load
                # is needed at sc=1 (l=0) and every even sc>=2 (l advances)
                if sc == 1 or (sc >= 2 and sc % 2 == 0):
                    x8t = x8pool.tile([P, 3, ND, SCL], FP8, tag="x8")
                    nc.sync.dma_start(
                        out=x8t[:],
                        in_=xa8[sc // 2, :, :].rearrange(
                            "p (i j s) -> p i j s", i=3, j=ND
                        ),
                    )
                g = sc % (SCO // SC)  # position within the out-store group
                if g == 0:
                    for ec in range(N_EC):
                        otiles[ec] = opool.tile(
                            [P, SCO], FP16, tag=f"ot{ec}", name=f"ot{ec}"
                        )
                ckg = cpool.tile([P, N_EC * SC], FP16, tag="ckg")
                ckvg = cpool.tile([P, N_EC * SC], FP16, tag="ckvg")
                deng = dpool.tile([P, N_EC * SC], FP16, tag="deng")
                t1s = [emit_unit(sc, ec, x8t, ckg, ckvg) for ec in range(N_EC)]
                # 1/ck for all 4 ec blocks of this s-chunk
                if den_mode == "lnexp":
                    lnk = dpool.tile([P, N_EC * SC], FP16, tag="lnk")
                    nc.scalar.activation(lnk[:], ckg[:], AF.Ln)
                    nc.scalar.activation(deng[:], lnk[:], AF.Exp, scale=-1.0)
                else:
                    with nc.allow_low_precision(reason="1/k_prefix; 5e-4 rel ok"):
                        nc.vector.reciprocal(deng[:], ckg[:])
                for ec in range(N_EC):
                    cs = slice(ec * SC, (ec + 1) * SC)
                    nc.vector.tensor_mul(
                        otiles[ec][:, g * SC : (g + 1) * SC], t1s[ec], deng[:, cs]
                    )
                if g == SCO // SC - 1:
                    for ec in range(N_EC):
                        nc.sync.dma_start(
                            out=outp[
                                ec * P : (ec + 1) * P,
                                (sc - g) * SC : (sc + 1) * SC,
                            ],
                            in_=otiles[ec][:],
                        )
                carry_k[0], carry_kv[0] = ckg, ckvg

        if repeat == 1:
            main_body()
        else:
            with tc.For_i(0, repeat, 1):
                main_body()

    nc.compile()
    return nc


def _e4m3(x):
    return np.clip(x, -240, 240).astype(ml_dtypes.float8_e4m3)


def _host_prep(v, k, q, Wq, Wk, Wv):
    """Build the 8 per-core input maps (packed, per-partition contiguous)."""
    xa8_b, xb16_b = {}, {}
    for b in range(B):
        a8 = np.empty((NL, P, 3, ND, SCL), dtype=ml_dtypes.float8_e4m3)
        b16 = np.empty((P, 3, ND, SC), dtype=np.float16)
        for i, x in enumerate((q, k, v)):
            # x[b]: [S, D] -> chunks [NL, SCL, ND, P] -> [NL, P, ND, SCL]
            x8 = _e4m3(np.asarray(x[b], np.float32))
            a8[:, :, i] = x8.reshape(NL, SCL, ND, P).transpose(0, 3, 2, 1)
            b16[:, i] = (
                np.asarray(x[b][:SC], np.float32)
                .astype(np.float16)
                .reshape(SC, ND, P)
                .transpose(2, 1, 0)
            )
        xa8_b[b] = a8.reshape(NL, P, 3 * ND * SCL)
        xb16_b[b] = b16.reshape(P, 3 * ND * SC)
    in_maps = []
    for c in range(NCORES):
        b, h = c // 2, c % 2
        e0 = h * EH
        w16 = np.empty((P, 3, ND, EH), dtype=np.float16)
        w8 = np.empty((P, 3, ND, EH), dtype=ml_dtypes.float8_e4m3)
        for i, W in enumerate((Wq, Wk, Wv)):
            # W^T[:, e0:e0+EH]: [D, EH] -> [ND, P, EH] -> [P, ND, EH]
            wt = np.ascontiguousarray(np.asarray(W, np.float32).T[:, e0 : e0 + EH])
            w16[:, i] = wt.astype(np.float16).reshape(ND, P, EH).transpose(1, 0, 2)
            w8[:, i] = _e4m3(wt * WS).reshape(ND, P, EH).transpose(1, 0, 2)
        in_maps.append(
            {
                "xa8": xa8_b[b],
                "xb16": xb16_b[b],
                "wb16": w16.reshape(P, 3 * ND * EH),
                "wb8": w8.reshape(P, 3 * ND * EH),
            }
        )
    return in_maps


_NC_CACHE = None


def _get_nc():
    global _NC_CACHE
    if _NC_CACHE is None:
        _NC_CACHE = build_nc()
    return _NC_CACHE


def run_spmd(v, k, q, Wq, Wk, Wv, **kwargs):
    """Run on 8 cores; returns (assembled output [B,S,E] fp32, raw results)."""
    nc = _get_nc()
    in_maps = _host_prep(v, k, q, Wq, Wk, Wv)
    res = run_bass_kernel_spmd(nc, in_maps, core_ids=list(range(NCORES)), **kwargs)
    full = np.empty((B, S, E), dtype=np.float32)
    for c in range(NCORES):
        b, h = c // 2, c % 2
        full[b, :, h * EH : (h + 1) * EH] = res.results[c]["out"].T.astype(np.float32)
    return full, res


def kernel(v, k, q, Wq, Wk, Wv):
    v, k, q, Wq, Wk, Wv = (
        np.asarray(a, dtype=np.float32) for a in (v, k, q, Wq, Wk, Wv)
    )
    full, _ = run_spmd(v, k, q, Wq, Wk, Wv)
    return full


# revision 32
# speedup vs baseline: 12.6731x; 1.5523x over previous
"""Trainium2 Bass kernel for nn_AttnFreeLayer (linear-attention-style layer).

Computes, for inputs q,k,v [B,S,D] and weights Wq,Wk,Wv [E,D] (E=D):
    q_in = elu(q @ Wq^T) + 1
    k_in = elu(k @ Wk^T) + 1
    v_in = v @ Wv^T
    kv_in = k_in * v_in
    out = q_in * (kv_in + cumsum_s(kv_in)) / cumsum_s(k_in)

Sharding: 8 cores = 4 batches x 2 halves of the output dim E; no
collectives. Each core computes out[b, e0:e0+512, :] in a TRANSPOSED
[e, s] layout: the projection matmuls put W chunks stationary and x^T
moving, so outputs land with e on partitions and s on the free dim.
The seq-cumsum runs along the free dimension via the DVE
tensor_tensor_scan instruction (chained across s-chunks through its
`initial` operand).

v2 restructure vs the original baseline:
- DMA coalescing: q/k/v packed into ONE DRAM tensor laid out so each
  load is a single 24KB-per-partition contiguous transfer (8 x-loads
  for the whole pass instead of 45), and output is staged in SBUF and
  stored 2048 columns at a time (16 stores instead of 64). 27 DMAs
  total vs 118.
- elu(x)+1 = min(exp(x), relu(x)+1)  (e^x >= 1+x everywhere, equality
  only at 0, so the min selects exp(x) for x<0 and x+1 for x>0).
  That needs only ONE table-lookup pass per element on ACT (exp)
  plus a relu pass, vs the 3-pass exp(-relu(-x))+relu(x) chain.
  The +1 runs on DVE tensor_scalar (4x rate), min on tensor_tensor
  (2x rate).
- den_mode="lnexp": 1/k_prefix = exp(-ln(k_prefix)) on ACT (exp, ln,
  relu and copy all live in the SAME activation table set, so no
  table-swap penalty), batched [P, 4*SC] per s-chunk.  Moves the
  1x-rate DVE reciprocal onto the ACT engine. den_mode="recip" keeps
  the DVE reciprocal.

Precision: hybrid fp16/fp8 as before. First 512 seq positions use
fp16 matmuls, the remaining 7680 use fp8-e4m3 DoubleRow matmuls.
fp8 weights pre-scaled by 16 (avoids e4m3 subnormals); downstream ACT
ops undo it via their free `scale`, the DVE relu+1 path folds it into
the relu tensor_scalar.
"""

import sys

for _p in ("/opt/trn_rl_repo",):
    if _p not in sys.path:
        sys.path.insert(0, _p)

from contextlib import ExitStack

import numpy as np
import ml_dtypes

import concourse.bass as bass
import concourse.tile as tile
from concourse import bacc
from concourse import mybir
from concourse.alu_op_type import AluOpType
from concourse.bass_utils import run_bass_kernel_spmd

FP8 = mybir.dt.float8e4
FP16 = mybir.dt.float16
FP32 = mybir.dt.float32
AF = mybir.ActivationFunctionType
DR = mybir.MatmulPerfMode.DoubleRow

# --- custom fused DVE op: t1 = q1 * (kv + carry + cumsum(kv)) --------------
# Registered via the documented extension point (append to dve_ops.OPS);
# the per-NEFF DVE table is generated from the registry by name, and the
# uops sha is computed here the same way DveOp.compile() checks it.
from concourse import dve_ops as _dops
from concourse import dve_spec as _dspec
from concourse.dve_uop import DveOpSpec as _DveOpSpec


def _ref_t1_fused(in0, in1, s0, s1, imm2):
    return (
        in0.astype(np.float32)
        * (in1 + s0 + np.cumsum(in1.astype(np.float32), axis=-1))
    ).astype(np.float32)


def _register_t1_fused():
    name = "ANT_T1_CUMSUM_FUSED"
    for o in _dops.OPS:
        if o.name == name:
            return o
    body = _dspec.Src0 * (
        _dspec.Src1 + _dspec.scan(_dspec.AluOp.ADD, _dspec.Src1, init=_dspec.C0)
    )
    spec = _dspec.Spec(body=body, reference=_ref_t1_fused)
    row = _dops._CUSTOM_DVE_ROW_BASE + len(_dops.OPS)
    shas = {}
    for ver in ("v3", "v4"):
        uops = _dspec.lower(spec, ver=ver)
        shas[ver] = _DveOpSpec(
            name=name, opcode=row, uops=uops, rd1_en=_dspec._has_src1(spec)
        ).sha(ver)
    op = _dops.DveOp(name, spec, subdim=False, uops_sha=shas)
    _dops.OPS.append(op)
    _dops.CUSTOM_DVE_SPECS[name] = spec
    _dops._SUB_OPCODE_FOR_NAME[name] = row
    return op


T1_FUSED = _register_t1_fused()


def _register_dve_op(name, spec):
    for o in _dops.OPS:
        if o.name == name:
            return o
    row = _dops._CUSTOM_DVE_ROW_BASE + len(_dops.OPS)
    shas = {}
    for ver in ("v3", "v4"):
        uops = _dspec.lower(spec, ver=ver)
        shas[ver] = _DveOpSpec(
            name=name, opcode=row, uops=uops, rd1_en=_dspec._has_src1(spec)
        ).sha(ver)
    op = _dops.DveOp(name, spec, subdim=False, uops_sha=shas)
    _dops.OPS.append(op)
    _dops.CUSTOM_DVE_SPECS[name] = spec
    _dops._SUB_OPCODE_FOR_NAME[name] = row
    return op


def _ref_t1_fused_b(in0, in1, s0, s1, imm2):
    return (
        in0.astype(np.float32)
        * (in1 + s0 + np.cumsum(in1.astype(np.float32), axis=-1))
    ).astype(np.float32)


# variant B: carry rides as a body-side constant add (scan init stays the
# ADD identity) — same value as T1_FUSED, different uop schedule
T1_FUSED_B = _register_dve_op(
    "ANT_T1_CUMSUM_FUSED_B",
    _dspec.Spec(
        body=_dspec.Src0
        * (
            _dspec.Src1
            + _dspec.C0
            + _dspec.scan(_dspec.AluOp.ADD, _dspec.Src1, init=_dspec.Zero)
        ),
        reference=_ref_t1_fused_b,
    ),
)


def _ref_ck_den(in0, in1, s0, s1, imm2):
    ck = s0 + np.cumsum(in0.astype(np.float32), axis=-1)
    not_x = (~ck.astype(np.float32).view(np.int32)).view(np.float32)
    y0 = not_x * np.float32(s1)
    return (y0 * (np.float32(imm2) - ck * y0)).astype(np.float32)


def _make_ck_den_spec():
    ck = _dspec.C0 + _dspec.scan(_dspec.AluOp.ADD, _dspec.Src0, init=_dspec.Zero)
    nx = _dspec.Bin(_dspec.AluOp.BITWISE_NOT, ck, ck)
    y0 = nx * _dspec.C1
    return _dspec.Spec(body=y0 * (_dspec.C2 - ck * y0), reference=_ref_ck_den)


# den = approx-1/(carry + cumsum(k1)): BITWISE_NOT seed + ONE inline NR
# pass (max rel err 1.73e-3 on [1e-3, 3e4] with the stock Chebyshev pair;
# checked numerically) — replaces the native k-scan AND the reciprocal
CK_DEN_CONSTS = {"s1": -0.23549792, "imm2": 2.0017324}
CK_DEN = _register_dve_op("ANT_CK_DEN_FUSED", _make_ck_den_spec())

B, S, D, E = 4, 8192, 1024, 1024
NCORES = 8
EH = E // 2  # e-half per core
P = 128  # partition block
SC = 512  # s-chunk width (PSUM bank = 512 fp32)
N_SC = S // SC  # 16
N_EC = EH // P  # 4
ND = D // P  # 8 contraction chunks
WS = 16.0  # fp8 weight prescale
SCL = 1024  # x-load width (2 s-chunks per DMA)
NL = S // SCL  # 8 loads
SCO = 2048  # out-store width (4 s-chunks per DMA)


def build_nc(
    repeat=1,
    debug=False,
    den_mode="recipfast",
    elu_mode="act3",
    fuse_t1=False,
    scan_eng="vector",
    gps=False,
    probe="none",
):
    """den_mode: "recipfast" (RECIPROCAL_APPROX_FAST custom DVE op, 1
    cyc/elem vs ~6 for InstReciprocal on HW; needs fp32 ck), "recip"
    (InstReciprocal), or "lnexp" (exp(-ln(ck)) on ACT).
    elu_mode: "act3" = exp(-relu(-x))+relu(x) via 3 ACT passes + one 2x
    DVE add (ACT has headroom; DVE is critical). "minexp" =
    min(exp(x), relu(x)+1): one fewer ACT pass but +1 DVE op.
    fuse_t1: use the ANT_T1_CUMSUM_FUSED custom DVE op
    (t1 = q1*(kv + carry + cumsum(kv)) in ONE 1x pass, replacing
    scan_kv + num-add + t1-mul), with the kv chunk-sum carried by
    tensor_tensor_reduce's free accumulator.
    scan_eng: must be "vector" — TensorTensorScanArith is NOT a valid
    opcode on the Pool engine (walrus codegen asserts).
    probe: timing-only variants with WRONG numerics — "noscan",
    "norecip", "noelu", "nomm"."""
    nc = bacc.Bacc("TRN2", target_bir_lowering=False, debug=debug)

    # packed inputs: per-partition contiguous 24KB loads
    xa8 = nc.declare_dram_parameter("xa8", [NL, P, 3 * ND * SCL], FP8, isOutput=False)
    xb16 = nc.declare_dram_parameter("xb16", [P, 3 * ND * SC], FP16, isOutput=False)
    wb16 = nc.declare_dram_parameter("wb16", [P, 3 * ND * EH], FP16, isOutput=False)
    wb8 = nc.declare_dram_parameter("wb8", [P, 3 * ND * EH], FP8, isOutput=False)
    outp = nc.declare_dram_parameter("out", [EH, S], FP16, isOutput=True)

    with tile.TileContext(nc) as tc, ExitStack() as ctx:
        wpool = ctx.enter_context(tc.tile_pool(name="w", bufs=1))
        x8pool = ctx.enter_context(tc.tile_pool(name="x8", bufs=2))
        apool = ctx.enter_context(tc.tile_pool(name="act", bufs=2))
        vpool = ctx.enter_context(tc.tile_pool(name="vv", bufs=2))
        kvpool = ctx.enter_context(tc.tile_pool(name="kv", bufs=3))
        cpool = ctx.enter_context(tc.tile_pool(name="cum", bufs=2))
        dpool = ctx.enter_context(tc.tile_pool(name="den", bufs=2))
        opool = ctx.enter_context(tc.tile_pool(name="out", bufs=2))
        pp = ctx.enter_context(tc.tile_pool(name="pqk", bufs=3, space="PSUM"))
        pvp = ctx.enter_context(tc.tile_pool(name="pv", bufs=2, space="PSUM"))

        # --- resident weights + first-chunk fp16 x ---
        w16_t = wpool.tile([P, 3, ND, EH], FP16, tag="w16")
        nc.sync.dma_start(
            out=w16_t[:], in_=wb16[:].rearrange("p (i j e) -> p i j e", i=3, j=ND)
        )
        w8_t = wpool.tile([P, 3, ND, EH], FP8, tag="w8")
        nc.sync.dma_start(
            out=w8_t[:], in_=wb8[:].rearrange("p (i j e) -> p i j e", i=3, j=ND)
        )
        x16_t = wpool.tile([P, 3, ND, SC], FP16, tag="x16")
        nc.sync.dma_start(
            out=x16_t[:], in_=xb16[:].rearrange("p (i j s) -> p i j s", i=3, j=ND)
        )

        ck_dt = FP32 if den_mode == "recipfast" else FP16

        def main_body():
            carry_k = [None]  # [P, 4*SC] group tile of prev sc (or None)
            carry_kv = [None]  # fused: [P, N_EC] fp32 chunk-sum carries
            otiles = [None] * N_EC

            def emit_unit(sc, ec, x8t, ckg, ckvg, kvc):
                # fused mode: kvc = [P, N_EC] fp32 chunk-sum carry tile;
                # non-fused: ckvg = [P, N_EC*SC] cumsum group tile
                fp8 = sc > 0
                e0 = ec * P
                sscale = (1.0 / WS) if fp8 else 1.0
                half = (sc % 2) if fp8 else 0
                pqk = pp.tile([P, 2 * SC], FP32, tag="pqk")
                pv_ = pvp.tile([P, SC], FP32, tag="pv")
                for i in range(0 if probe != "nomm" else 3, 3):
                    dst = pqk[:, i * SC : (i + 1) * SC] if i < 2 else pv_[:]
                    if fp8:
                        for j in range(ND // 2):
                            nc.tensor.matmul(
                                dst,
                                lhsT=w8_t[:, i, 2 * j : 2 * j + 2, e0 : e0 + P],
                                rhs=x8t[
                                    :,
                                    i,
                                    2 * j : 2 * j + 2,
                                    half * SC : (half + 1) * SC,
                                ],
                                start=(j == 0),
                                stop=(j == ND // 2 - 1),
                                perf_mode=DR,
                                skip_group_check=True,
                            )
                    else:
                        for j in range(ND):
                            nc.tensor.matmul(
                                dst,
                                lhsT=w16_t[:, i, j, e0 : e0 + P],
                                rhs=x16_t[:, i, j, :],
                                start=(j == 0),
                                stop=(j == ND - 1),
                                skip_group_check=True,
                            )
                # elu(x)+1
                v1 = vpool.tile([P, SC], FP16, tag="v1")
                nc.scalar.activation(v1[:], pv_[:], AF.Copy, scale=sscale)
                qk1 = apool.tile([P, 2 * SC], FP16, tag="qk1")
                if probe == "noelu":
                    nc.scalar.activation(qk1[:], pqk[:], AF.Copy, scale=sscale)
                elif elu_mode == "act3":
                    # exp(-relu(-x)) + relu(x): 3 ACT table passes + one
                    # 2x-rate DVE add — minimal DVE load
                    rn = apool.tile([P, 2 * SC], FP16, tag="rn")
                    nc.scalar.activation(rn[:], pqk[:], AF.Relu, scale=-sscale)
                    ex = apool.tile([P, 2 * SC], FP16, tag="ex")
                    nc.scalar.activation(ex[:], rn[:], AF.Exp, scale=-1.0)
                    rp = apool.tile([P, 2 * SC], FP16, tag="rp")
                    nc.scalar.activation(rp[:], pqk[:], AF.Relu, scale=sscale)
                    nc.vector.tensor_add(qk1[:], ex[:], rp[:])
                else:  # minexp
                    ex = apool.tile([P, 2 * SC], FP16, tag="ex")
                    nc.scalar.activation(ex[:], pqk[:], AF.Exp, scale=sscale)
                    rp = apool.tile([P, 2 * SC], FP16, tag="rp")
                    nc.scalar.activation(rp[:], pqk[:], AF.Relu, scale=sscale)
                    rp1 = apool.tile([P, 2 * SC], FP16, tag="rp1")
                    nc.vector.tensor_scalar_add(rp1[:], rp[:], 1.0)
                    nc.vector.tensor_tensor(qk1[:], ex[:], rp1[:], AluOpType.min)
                q1 = qk1[:, 0:SC]
                k1 = qk1[:, SC : 2 * SC]
                cs = slice(ec * SC, (ec + 1) * SC)
                ik = 0.0 if sc == 0 else carry_k[0][:, (ec + 1) * SC - 1 : (ec + 1) * SC]
                seng = nc.gpsimd if scan_eng == "gpsimd" else nc.vector
                if probe == "noscan":
                    nc.vector.tensor_copy(ckg[:, cs], k1)
                else:
                    seng.tensor_tensor_scan(
                        ckg[:, cs], k1, k1, ik, op0=AluOpType.add, op1=AluOpType.bypass
                    )
                kv = kvpool.tile([P, SC], FP16, tag="kv")
                t1 = kvpool.tile([P, SC], FP16, tag="t1")
                if fuse_t1:
                    ikv = 0.0 if sc == 0 else carry_kv[0][:, ec : ec + 1]
                    # kv = k1*v1, and (for free) kvc[:, ec] = ikv + sum(kv)
                    nc.vector.tensor_tensor_reduce(
                        out=kv[:],
                        in0=k1,
                        in1=v1[:],
                        scale=1.0,
                        scalar=ikv,
                        op0=AluOpType.mult,
                        op1=AluOpType.add,
                        accum_out=kvc[:, ec : ec + 1],
                    )
                    # t1 = q1 * (kv + ikv + cumsum(kv)) in ONE DVE pass
                    nc.vector._custom_dve(
                        T1_FUSED, out=t1[:], in0=q1, in1=kv[:], s0=ikv
                    )
                else:
                    (nc.gpsimd if gps else nc.vector).tensor_mul(kv[:], k1, v1[:])
                    ikv = (
                        0.0
                        if sc == 0
                        else carry_kv[0][:, (ec + 1) * SC - 1 : (ec + 1) * SC]
                    )
                    if probe == "noscan":
                        nc.vector.tensor_copy(ckvg[:, cs], kv[:])
                    else:
                        seng.tensor_tensor_scan(
                            ckvg[:, cs],
                            kv[:],
                            kv[:],
                            ikv,
                            op0=AluOpType.add,
                            op1=AluOpType.bypass,
                        )
                    num = kvpool.tile([P, SC], FP16, tag="num")
                    (nc.gpsimd if gps else nc.vector).tensor_add(
                        num[:], ckvg[:, cs], kv[:]
                    )
                    nc.vector.tensor_mul(t1[:], q1, num[:])
                return t1

            x8t = None
            for sc in range(N_SC):
                # fp8 chunk sc lives in load l=sc//2, half sc%2; a new load
                # is needed at sc=1 (l=0) and every even sc>=2 (l advances)
                if sc == 1 or (sc >= 2 and sc % 2 == 0):
                    x8t = x8pool.tile([P, 3, ND, SCL], FP8, tag="x8")
                    nc.sync.dma_start(
                        out=x8t[:],
                        in_=xa8[sc // 2, :, :].rearrange(
                            "p (i j s) -> p i j s", i=3, j=ND
                        ),
                    )
                g = sc % (SCO // SC)  # position within the out-store group
                if g == 0:
                    for ec in range(N_EC):
                        otiles[ec] = opool.tile(
                            [P, SCO], FP16, tag=f"ot{ec}", name=f"ot{ec}"
                        )
                ckg = cpool.tile([P, N_EC * SC], ck_dt, tag="ckg")
                ckvg = (
                    None
                    if fuse_t1
                    else cpool.tile([P, N_EC * SC], FP16, tag="ckvg", name="ckvg")
                )
                kvc = (
                    cpool.tile([P, N_EC], FP32, tag="kvc", bufs=3, name="kvc")
                    if fuse_t1
                    else None
                )
                deng = dpool.tile([P, N_EC * SC], FP16, tag="deng")
                t1s = [emit_unit(sc, ec, x8t, ckg, ckvg, kvc) for ec in range(N_EC)]
                # 1/ck for all 4 ec blocks of this s-chunk
                if probe == "norecip":
                    nc.vector.tensor_copy(deng[:], ckg[:])
                elif den_mode == "recipfast":
                    c = _dops.RECIP_APPROX_FAST_CONSTS
                    # fp16 out (wrapper wants fp32/fp32; the bit-trick is on
                    # the INPUT read path — fp32 ckg — and the output write
                    # converts); validated against the reference on HW
                    nc.vector._custom_dve(
                        _dops.RECIPROCAL_APPROX_FAST,
                        out=deng[:],
                        in0=ckg[:],
                        s0=c["s0"],
                        s1=c["s1"],
                        imm2=c["imm2"],
                    )
                elif den_mode == "lnexp":
                    lnk = dpool.tile([P, N_EC * SC], FP16, tag="lnk")
                    nc.scalar.activation(lnk[:], ckg[:], AF.Ln)
                    nc.scalar.activation(deng[:], lnk[:], AF.Exp, scale=-1.0)
                else:
                    with nc.allow_low_precision(reason="1/k_prefix; 5e-4 rel ok"):
                        nc.vector.reciprocal(deng[:], ckg[:])
                for ec in range(N_EC):
                    cs = slice(ec * SC, (ec + 1) * SC)
                    nc.vector.tensor_mul(
                        otiles[ec][:, g * SC : (g + 1) * SC], t1s[ec], deng[:, cs]
                    )
                if g == SCO // SC - 1:
                    for ec in range(N_EC):
                        nc.sync.dma_start(
                            out=outp[
                                ec * P : (ec + 1) * P,
                                (sc - g) * SC : (sc + 1) * SC,
                            ],
                            in_=otiles[ec][:],
                        )
                carry_k[0] = ckg
                carry_kv[0] = kvc if fuse_t1 else ckvg

        if repeat == 1:
            main_body()
        else:
            with tc.For_i(0, repeat, 1):
                main_body()

    nc.compile()
    return nc


def _e4m3(x):
    return np.clip(x, -240, 240).astype(ml_dtypes.float8_e4m3)


def _host_prep(v, k, q, Wq, Wk, Wv):
    """Build the 8 per-core input maps (packed, per-partition contiguous)."""
    xa8_b, xb16_b = {}, {}
    for b in range(B):
        a8 = np.empty((NL, P, 3, ND, SCL), dtype=ml_dtypes.float8_e4m3)
        b16 = np.empty((P, 3, ND, SC), dtype=np.float16)
        for i, x in enumerate((q, k, v)):
            # x[b]: [S, D] -> chunks [NL, SCL, ND, P] -> [NL, P, ND, SCL]
            x8 = _e4m3(np.asarray(x[b], np.float32))
            a8[:, :, i] = x8.reshape(NL, SCL, ND, P).transpose(0, 3, 2, 1)
            b16[:, i] = (
                np.asarray(x[b][:SC], np.float32)
                .astype(np.float16)
                .reshape(SC, ND, P)
                .transpose(2, 1, 0)
            )
        xa8_b[b] = a8.reshape(NL, P, 3 * ND * SCL)
        xb16_b[b] = b16.reshape(P, 3 * ND * SC)
    in_maps = []
    for c in range(NCORES):
        b, h = c // 2, c % 2
        e0 = h * EH
        w16 = np.empty((P, 3, ND, EH), dtype=np.float16)
        w8 = np.empty((P, 3, ND, EH), dtype=ml_dtypes.float8_e4m3)
        for i, W in enumerate((Wq, Wk, Wv)):
            # W^T[:, e0:e0+EH]: [D, EH] -> [ND, P, EH] -> [P, ND, EH]
            wt = np.ascontiguousarray(np.asarray(W, np.float32).T[:, e0 : e0 + EH])
            w16[:, i] = wt.astype(np.float16).reshape(ND, P, EH).transpose(1, 0, 2)
            w8[:, i] = _e4m3(wt * WS).reshape(ND, P, EH).transpose(1, 0, 2)
        in_maps.append(
            {
                "xa8": xa8_b[b],
                "xb16": xb16_b[b],
                "wb16": w16.reshape(P, 3 * ND * EH),
                "wb8": w8.reshape(P, 3 * ND * EH),
            }
        )
    return in_maps


_NC_CACHE = None


def _get_nc():
    global _NC_CACHE
    if _NC_CACHE is None:
        _NC_CACHE = build_nc()
    return _NC_CACHE


def run_spmd(v, k, q, Wq, Wk, Wv, **kwargs):
    """Run on 8 cores; returns (assembled output [B,S,E] fp32, raw results)."""
    nc = _get_nc()
    in_maps = _host_prep(v, k, q, Wq, Wk, Wv)
    res = run_bass_kernel_spmd(nc, in_maps, core_ids=list(range(NCORES)), **kwargs)
    full = np.empty((B, S, E), dtype=np.float32)
    for c in range(NCORES):
        b, h = c // 2, c % 2
        full[b, :, h * EH : (h + 1) * EH] = res.results[c]["out"].T.astype(np.float32)
    return full, res


def kernel(v, k, q, Wq, Wk, Wv):
    v, k, q, Wq, Wk, Wv = (
        np.asarray(a, dtype=np.float32) for a in (v, k, q, Wq, Wk, Wv)
    )
    full, _ = run_spmd(v, k, q, Wq, Wk, Wv)
    return full


# revision 40
# speedup vs baseline: 14.1541x; 1.1169x over previous
"""Trainium2 Bass kernel for nn_AttnFreeLayer (linear-attention-style layer).

Computes, for inputs q,k,v [B,S,D] and weights Wq,Wk,Wv [E,D] (E=D):
    q_in = elu(q @ Wq^T) + 1
    k_in = elu(k @ Wk^T) + 1
    v_in = v @ Wv^T
    kv_in = k_in * v_in
    out = q_in * (kv_in + cumsum_s(kv_in)) / cumsum_s(k_in)

Sharding: 8 cores = 4 batches x 2 halves of the output dim E; no
collectives. Each core computes out[b, e0:e0+512, :] in a TRANSPOSED
[e, s] layout: the projection matmuls put W chunks stationary and x^T
moving, so outputs land with e on partitions and s on the free dim.
The seq-cumsum runs along the free dimension via the DVE
tensor_tensor_scan instruction (chained across s-chunks through its
`initial` operand).

v3 (337us/pass measured, repeat-delta min-of-12, vs 574us harness
baseline; rel err 1.27e-3 vs the 2e-2 gate) vs the original baseline:
- DMA coalescing: q/k/v packed into ONE DRAM tensor laid out so each
  load is a single 24KB-per-partition contiguous transfer (8 x-loads
  for the whole pass instead of 45), and output is staged in SBUF and
  stored 2048 columns at a time (16 stores instead of 64). 27 DMAs
  total vs 118.
- den = 1/k_prefix via the RECIPROCAL_APPROX_FAST custom DVE op
  (BITWISE_NOT exponent-flip seed + 2 inline Newton passes, ~51 ULP),
  batched [P, 4*SC] per s-chunk with an fp32 k-scan feeding it and
  fp16 output.  Probe-measured: native InstReciprocal runs at ~6
  cycles/element on HW (~192us of the old 523us pass!); the custom op
  runs at 1 elem/cycle/partition.
- elu stays the 3-ACT-pass exp(-relu(-x))+relu(x) form: probe A/B
  showed DVE is the critical engine and ACT has headroom, so the
  min(exp(x), relu(x)+1) variant (fewer ACT passes, +1 DVE op) is a
  net LOSS; its flag remains for reference.
- Scans stay on DVE (TensorTensorScanArith is invalid on Pool; walrus
  asserts) and run at ~2 cycles/element on HW (no dve perf mode).
- tensor_tensor_reduce crashes this runtime (NRT_EXEC_UNIT_UNRECOVERABLE
  in a minimal 1-core repro), so the scan-fused custom ops registered
  below (T1_FUSED*/CK_DEN, which would fold cumsum+add+mul into one
  1x pass) are DISABLED by default (fuse_t1=False) pending a working
  carry path; RECIPROCAL_APPROX_FAST is the one custom op in use.

Precision: hybrid fp16/fp8 as before. First 512 seq positions use
fp16 matmuls, the remaining 7680 use fp8-e4m3 DoubleRow matmuls.
fp8 weights pre-scaled by 16 (avoids e4m3 subnormals); downstream ACT
ops undo it via their free `scale`.
"""

import sys

for _p in ("/opt/trn_rl_repo",):
    if _p not in sys.path:
        sys.path.insert(0, _p)

from contextlib import ExitStack

import numpy as np
import ml_dtypes

import concourse.bass as bass
import concourse.tile as tile
from concourse import bacc
from concourse import mybir
from concourse.alu_op_type import AluOpType
from concourse.bass_utils import run_bass_kernel_spmd

FP8 = mybir.dt.float8e4
FP16 = mybir.dt.float16
FP32 = mybir.dt.float32
AF = mybir.ActivationFunctionType
DR = mybir.MatmulPerfMode.DoubleRow

# --- custom fused DVE op: t1 = q1 * (kv + carry + cumsum(kv)) --------------
# Registered via the documented extension point (append to dve_ops.OPS);
# the per-NEFF DVE table is generated from the registry by name, and the
# uops sha is computed here the same way DveOp.compile() checks it.
from concourse import dve_ops as _dops
from concourse import dve_spec as _dspec
from concourse.dve_uop import DveOpSpec as _DveOpSpec


def _ref_t1_fused(in0, in1, s0, s1, imm2):
    return (
        in0.astype(np.float32)
        * (in1 + s0 + np.cumsum(in1.astype(np.float32), axis=-1))
    ).astype(np.float32)


def _register_t1_fused():
    name = "ANT_T1_CUMSUM_FUSED"
    for o in _dops.OPS:
        if o.name == name:
            return o
    body = _dspec.Src0 * (
        _dspec.Src1 + _dspec.scan(_dspec.AluOp.ADD, _dspec.Src1, init=_dspec.C0)
    )
    spec = _dspec.Spec(body=body, reference=_ref_t1_fused)
    row = _dops._CUSTOM_DVE_ROW_BASE + len(_dops.OPS)
    shas = {}
    for ver in ("v3", "v4"):
        uops = _dspec.lower(spec, ver=ver)
        shas[ver] = _DveOpSpec(
            name=name, opcode=row, uops=uops, rd1_en=_dspec._has_src1(spec)
        ).sha(ver)
    op = _dops.DveOp(name, spec, subdim=False, uops_sha=shas)
    _dops.OPS.append(op)
    _dops.CUSTOM_DVE_SPECS[name] = spec
    _dops._SUB_OPCODE_FOR_NAME[name] = row
    return op


T1_FUSED = _register_t1_fused()


def _register_dve_op(name, spec):
    for o in _dops.OPS:
        if o.name == name:
            return o
    row = _dops._CUSTOM_DVE_ROW_BASE + len(_dops.OPS)
    shas = {}
    for ver in ("v3", "v4"):
        uops = _dspec.lower(spec, ver=ver)
        shas[ver] = _DveOpSpec(
            name=name, opcode=row, uops=uops, rd1_en=_dspec._has_src1(spec)
        ).sha(ver)
    op = _dops.DveOp(name, spec, subdim=False, uops_sha=shas)
    _dops.OPS.append(op)
    _dops.CUSTOM_DVE_SPECS[name] = spec
    _dops._SUB_OPCODE_FOR_NAME[name] = row
    return op


def _ref_t1_fused_b(in0, in1, s0, s1, imm2):
    return (
        in0.astype(np.float32)
        * (in1 + s0 + np.cumsum(in1.astype(np.float32), axis=-1))
    ).astype(np.float32)


# variant B: carry rides as a body-side constant add (scan init stays the
# ADD identity) — same value as T1_FUSED, different uop schedule
T1_FUSED_B = _register_dve_op(
    "ANT_T1_CUMSUM_FUSED_B",
    _dspec.Spec(
        body=_dspec.Src0
        * (
            _dspec.Src1
            + _dspec.C0
            + _dspec.scan(_dspec.AluOp.ADD, _dspec.Src1, init=_dspec.Zero)
        ),
        reference=_ref_t1_fused_b,
    ),
)


def _ref_ck_den(in0, in1, s0, s1, imm2):
    ck = s0 + np.cumsum(in0.astype(np.float32), axis=-1)
    not_x = (~ck.astype(np.float32).view(np.int32)).view(np.float32)
    y0 = not_x * np.float32(s1)
    return (y0 * (np.float32(imm2) - ck * y0)).astype(np.float32)


def _make_ck_den_spec():
    ck = _dspec.C0 + _dspec.scan(_dspec.AluOp.ADD, _dspec.Src0, init=_dspec.Zero)
    nx = _dspec.Bin(_dspec.AluOp.BITWISE_NOT, ck, ck)
    y0 = nx * _dspec.C1
    return _dspec.Spec(body=y0 * (_dspec.C2 - ck * y0), reference=_ref_ck_den)


# den = approx-1/(carry + cumsum(k1)): BITWISE_NOT seed + ONE inline NR
# pass (max rel err 1.73e-3 on [1e-3, 3e4] with the stock Chebyshev pair;
# checked numerically) — replaces the native k-scan AND the reciprocal
CK_DEN_CONSTS = {"s1": -0.23549792, "imm2": 2.0017324}
CK_DEN = _register_dve_op("ANT_CK_DEN_FUSED", _make_ck_den_spec())


def _ref_num_cumsum(in0, in1, s0, s1, imm2):
    return (in0 + s0 + np.cumsum(in0.astype(np.float32), axis=-1)).astype(np.float32)


# num = kv + carry + cumsum(kv): ONE 1x DVE pass replacing the 2cyc/elem
# native kv-scan plus the num tensor_add (the carry chain is recovered as
# num[-1] - kv[-1], a [P,1] subtract)
NUM_CUMSUM = _register_dve_op(
    "ANT_NUM_CUMSUM",
    _dspec.Spec(
        body=_dspec.Src0
        + _dspec.C0
        + _dspec.scan(_dspec.AluOp.ADD, _dspec.Src0, init=_dspec.Zero),
        reference=_ref_num_cumsum,
    ),
)


def _ref_ck_cumsum(in0, in1, s0, s1, imm2):
    return (s0 + np.cumsum(in0.astype(np.float32), axis=-1)).astype(np.float32)


# ck = carry + cumsum(k1): custom 1x pass replacing the 2cyc/elem native
# scan (carry still read from the materialized fp32 ckg's last column).
# DISABLED (ck_custom=False): crashes at runtime when the output AP is a
# free-dim-offset slice of a wider tile (NUM_CUMSUM with a full-tile out
# works); needs a dedicated per-ec tile to retry.
CK_CUMSUM = _register_dve_op(
    "ANT_CK_CUMSUM",
    _dspec.Spec(
        body=_dspec.C0 + _dspec.scan(_dspec.AluOp.ADD, _dspec.Src0, init=_dspec.Zero),
        reference=_ref_ck_cumsum,
    ),
)

B, S, D, E = 4, 8192, 1024, 1024
NCORES = 8
EH = E // 2  # e-half per core
P = 128  # partition block
SC = 512  # s-chunk width (PSUM bank = 512 fp32)
N_SC = S // SC  # 16
N_EC = EH // P  # 4
ND = D // P  # 8 contraction chunks
WS = 16.0  # fp8 weight prescale
SCL = 1024  # x-load width (2 s-chunks per DMA)
NL = S // SCL  # 8 loads
SCO = 2048  # out-store width (4 s-chunks per DMA)


def build_nc(
    repeat=1,
    debug=False,
    den_mode="recipfast",
    elu_mode="act3",
    fuse_t1=True,
    ck_custom=False,
    scan_eng="vector",
    gps=False,
    probe="none",
):
    """den_mode: "recipfast" (RECIPROCAL_APPROX_FAST custom DVE op, 1
    cyc/elem vs ~6 for InstReciprocal on HW; needs fp32 ck), "recip"
    (InstReciprocal), or "lnexp" (exp(-ln(ck)) on ACT).
    elu_mode: "act3" = exp(-relu(-x))+relu(x) via 3 ACT passes + one 2x
    DVE add (ACT has headroom; DVE is critical). "minexp" =
    min(exp(x), relu(x)+1): one fewer ACT pass but +1 DVE op.
    fuse_t1: use the ANT_T1_CUMSUM_FUSED custom DVE op
    (t1 = q1*(kv + carry + cumsum(kv)) in ONE 1x pass, replacing
    scan_kv + num-add + t1-mul), with the kv chunk-sum carried by
    tensor_tensor_reduce's free accumulator.
    scan_eng: must be "vector" — TensorTensorScanArith is NOT a valid
    opcode on the Pool engine (walrus codegen asserts).
    probe: timing-only variants with WRONG numerics — "noscan",
    "norecip", "noelu", "nomm"."""
    nc = bacc.Bacc("TRN2", target_bir_lowering=False, debug=debug)

    # packed inputs: per-partition contiguous 24KB loads
    xa8 = nc.declare_dram_parameter("xa8", [NL, P, 3 * ND * SCL], FP8, isOutput=False)
    xb16 = nc.declare_dram_parameter("xb16", [P, 3 * ND * SC], FP16, isOutput=False)
    wb16 = nc.declare_dram_parameter("wb16", [P, 3 * ND * EH], FP16, isOutput=False)
    wb8 = nc.declare_dram_parameter("wb8", [P, 3 * ND * EH], FP8, isOutput=False)
    outp = nc.declare_dram_parameter("out", [EH, S], FP16, isOutput=True)

    with tile.TileContext(nc) as tc, ExitStack() as ctx:
        wpool = ctx.enter_context(tc.tile_pool(name="w", bufs=1))
        x8pool = ctx.enter_context(tc.tile_pool(name="x8", bufs=2))
        apool = ctx.enter_context(tc.tile_pool(name="act", bufs=2))
        vpool = ctx.enter_context(tc.tile_pool(name="vv", bufs=2))
        kvpool = ctx.enter_context(tc.tile_pool(name="kv", bufs=3))
        cpool = ctx.enter_context(tc.tile_pool(name="cum", bufs=2))
        dpool = ctx.enter_context(tc.tile_pool(name="den", bufs=2))
        opool = ctx.enter_context(tc.tile_pool(name="out", bufs=2))
        pp = ctx.enter_context(tc.tile_pool(name="pqk", bufs=3, space="PSUM"))
        pvp = ctx.enter_context(tc.tile_pool(name="pv", bufs=2, space="PSUM"))

        # --- resident weights + first-chunk fp16 x ---
        w16_t = wpool.tile([P, 3, ND, EH], FP16, tag="w16")
        nc.sync.dma_start(
            out=w16_t[:], in_=wb16[:].rearrange("p (i j e) -> p i j e", i=3, j=ND)
        )
        w8_t = wpool.tile([P, 3, ND, EH], FP8, tag="w8")
        nc.sync.dma_start(
            out=w8_t[:], in_=wb8[:].rearrange("p (i j e) -> p i j e", i=3, j=ND)
        )
        x16_t = wpool.tile([P, 3, ND, SC], FP16, tag="x16")
        nc.sync.dma_start(
            out=x16_t[:], in_=xb16[:].rearrange("p (i j s) -> p i j s", i=3, j=ND)
        )

        ck_dt = FP32 if den_mode == "recipfast" else FP16

        def main_body():
            carry_k = [None]  # [P, 4*SC] group tile of prev sc (or None)
            carry_kv = [None]  # fused: [P, N_EC] fp32 chunk-sum carries
            otiles = [None] * N_EC

            def emit_unit(sc, ec, x8t, ckg, ckvg, kvc):
                # fused mode: kvc = [P, N_EC] fp32 chunk-sum carry tile;
                # non-fused: ckvg = [P, N_EC*SC] cumsum group tile
                fp8 = sc > 0
                e0 = ec * P
                sscale = (1.0 / WS) if fp8 else 1.0
                half = (sc % 2) if fp8 else 0
                pqk = pp.tile([P, 2 * SC], FP32, tag="pqk")
                pv_ = pvp.tile([P, SC], FP32, tag="pv")
                for i in range(0 if probe != "nomm" else 3, 3):
                    dst = pqk[:, i * SC : (i + 1) * SC] if i < 2 else pv_[:]
                    if fp8:
                        for j in range(ND // 2):
                            nc.tensor.matmul(
                                dst,
                                lhsT=w8_t[:, i, 2 * j : 2 * j + 2, e0 : e0 + P],
                                rhs=x8t[
                                    :,
                                    i,
                                    2 * j : 2 * j + 2,
                                    half * SC : (half + 1) * SC,
                                ],
                                start=(j == 0),
                                stop=(j == ND // 2 - 1),
                                perf_mode=DR,
                                skip_group_check=True,
                            )
                    else:
                        for j in range(ND):
                            nc.tensor.matmul(
                                dst,
                                lhsT=w16_t[:, i, j, e0 : e0 + P],
                                rhs=x16_t[:, i, j, :],
                                start=(j == 0),
                                stop=(j == ND - 1),
                                skip_group_check=True,
                            )
                # elu(x)+1
                v1 = vpool.tile([P, SC], FP16, tag="v1")
                nc.scalar.activation(v1[:], pv_[:], AF.Copy, scale=sscale)
                qk1 = apool.tile([P, 2 * SC], FP16, tag="qk1")
                if probe == "noelu":
                    nc.scalar.activation(qk1[:], pqk[:], AF.Copy, scale=sscale)
                elif elu_mode == "act3":
                    # exp(-relu(-x)) + relu(x): 3 ACT table passes + one
                    # 2x-rate DVE add — minimal DVE load
                    rn = apool.tile([P, 2 * SC], FP16, tag="rn")
                    nc.scalar.activation(rn[:], pqk[:], AF.Relu, scale=-sscale)
                    ex = apool.tile([P, 2 * SC], FP16, tag="ex")
                    nc.scalar.activation(ex[:], rn[:], AF.Exp, scale=-1.0)
                    rp = apool.tile([P, 2 * SC], FP16, tag="rp")
                    nc.scalar.activation(rp[:], pqk[:], AF.Relu, scale=sscale)
                    nc.vector.tensor_add(qk1[:], ex[:], rp[:])
                else:  # minexp
                    ex = apool.tile([P, 2 * SC], FP16, tag="ex")
                    nc.scalar.activation(ex[:], pqk[:], AF.Exp, scale=sscale)
                    rp = apool.tile([P, 2 * SC], FP16, tag="rp")
                    nc.scalar.activation(rp[:], pqk[:], AF.Relu, scale=sscale)
                    rp1 = apool.tile([P, 2 * SC], FP16, tag="rp1")
                    nc.vector.tensor_scalar_add(rp1[:], rp[:], 1.0)
                    nc.vector.tensor_tensor(qk1[:], ex[:], rp1[:], AluOpType.min)
                q1 = qk1[:, 0:SC]
                k1 = qk1[:, SC : 2 * SC]
                cs = slice(ec * SC, (ec + 1) * SC)
                ik = 0.0 if sc == 0 else carry_k[0][:, (ec + 1) * SC - 1 : (ec + 1) * SC]
                seng = nc.gpsimd if scan_eng == "gpsimd" else nc.vector
                if probe == "noscan":
                    nc.vector.tensor_copy(ckg[:, cs], k1)
                elif ck_custom:
                    nc.vector._custom_dve(CK_CUMSUM, out=ckg[:, cs], in0=k1, s0=ik)
                else:
                    seng.tensor_tensor_scan(
                        ckg[:, cs], k1, k1, ik, op0=AluOpType.add, op1=AluOpType.bypass
                    )
                kv = kvpool.tile([P, SC], FP16, tag="kv")
                t1 = kvpool.tile([P, SC], FP16, tag="t1")
                if fuse_t1:
                    nc.vector.tensor_mul(kv[:], k1, v1[:])
                    ikv = 0.0 if sc == 0 else carry_kv[0][:, ec : ec + 1]
                    num = kvpool.tile([P, SC], FP16, tag="num")
                    nc.vector._custom_dve(NUM_CUMSUM, out=num[:], in0=kv[:], s0=ikv)
                    # inclusive ckv carry for the next chunk
                    nc.vector.tensor_tensor(
                        kvc[:, ec : ec + 1],
                        num[:, SC - 1 : SC],
                        kv[:, SC - 1 : SC],
                        AluOpType.subtract,
                    )
                    nc.vector.tensor_mul(t1[:], q1, num[:])
                else:
                    (nc.gpsimd if gps else nc.vector).tensor_mul(kv[:], k1, v1[:])
                    ikv = (
                        0.0
                        if sc == 0
                        else carry_kv[0][:, (ec + 1) * SC - 1 : (ec + 1) * SC]
                    )
                    if probe == "noscan":
                        nc.vector.tensor_copy(ckvg[:, cs], kv[:])
                    else:
                        seng.tensor_tensor_scan(
                            ckvg[:, cs],
                            kv[:],
                            kv[:],
                            ikv,
                            op0=AluOpType.add,
                            op1=AluOpType.bypass,
                        )
                    num = kvpool.tile([P, SC], FP16, tag="num")
                    (nc.gpsimd if gps else nc.vector).tensor_add(
                        num[:], ckvg[:, cs], kv[:]
                    )
                    nc.vector.tensor_mul(t1[:], q1, num[:])
                return t1

            x8t = None
            for sc in range(N_SC):
                # fp8 chunk sc lives in load l=sc//2, half sc%2; a new load
                # is needed at sc=1 (l=0) and every even sc>=2 (l advances)
                if sc == 1 or (sc >= 2 and sc % 2 == 0):
                    x8t = x8pool.tile([P, 3, ND, SCL], FP8, tag="x8")
                    nc.sync.dma_start(
                        out=x8t[:],
                        in_=xa8[sc // 2, :, :].rearrange(
                            "p (i j s) -> p i j s", i=3, j=ND
                        ),
                    )
                g = sc % (SCO // SC)  # position within the out-store group
                if g == 0:
                    for ec in range(N_EC):
                        otiles[ec] = opool.tile(
                            [P, SCO], FP16, tag=f"ot{ec}", name=f"ot{ec}"
                        )
                ckg = cpool.tile([P, N_EC * SC], ck_dt, tag="ckg")
                ckvg = (
                    None
                    if fuse_t1
                    else cpool.tile([P, N_EC * SC], FP16, tag="ckvg", name="ckvg")
                )
                kvc = (
                    cpool.tile([P, N_EC], FP32, tag="kvc", bufs=3, name="kvc")
                    if fuse_t1
                    else None
                )
                deng = dpool.tile([P, N_EC * SC], FP16, tag="deng")
                t1s = [emit_unit(sc, ec, x8t, ckg, ckvg, kvc) for ec in range(N_EC)]
                # 1/ck for all 4 ec blocks of this s-chunk
                if probe == "norecip":
                    nc.vector.tensor_copy(deng[:], ckg[:])
                elif den_mode == "recipfast":
                    c = _dops.RECIP_APPROX_FAST_CONSTS
                    # fp16 out (wrapper wants fp32/fp32; the bit-trick is on
                    # the INPUT read path — fp32 ckg — and the output write
                    # converts); validated against the reference on HW
                    nc.vector._custom_dve(
                        _dops.RECIPROCAL_APPROX_FAST,
                        out=deng[:],
                        in0=ckg[:],
                        s0=c["s0"],
                        s1=c["s1"],
                        imm2=c["imm2"],
                    )
                elif den_mode == "lnexp":
                    lnk = dpool.tile([P, N_EC * SC], FP16, tag="lnk")
                    nc.scalar.activation(lnk[:], ckg[:], AF.Ln)
                    nc.scalar.activation(deng[:], lnk[:], AF.Exp, scale=-1.0)
                else:
                    with nc.allow_low_precision(reason="1/k_prefix; 5e-4 rel ok"):
                        nc.vector.reciprocal(deng[:], ckg[:])
                for ec in range(N_EC):
                    cs = slice(ec * SC, (ec + 1) * SC)
                    nc.vector.tensor_mul(
                        otiles[ec][:, g * SC : (g + 1) * SC], t1s[ec], deng[:, cs]
                    )
                if g == SCO // SC - 1:
                    for ec in range(N_EC):
                        nc.sync.dma_start(
                            out=outp[
                                ec * P : (ec + 1) * P,
                                (sc - g) * SC : (sc + 1) * SC,
                            ],
                            in_=otiles[ec][:],
                        )
                carry_k[0] = ckg
                carry_kv[0] = kvc if fuse_t1 else ckvg

        if repeat == 1:
            main_body()
        else:
            with tc.For_i(0, repeat, 1):
                main_body()

    nc.compile()
    return nc


def _e4m3(x):
    return np.clip(x, -240, 240).astype(ml_dtypes.float8_e4m3)


def _host_prep(v, k, q, Wq, Wk, Wv):
    """Build the 8 per-core input maps (packed, per-partition contiguous)."""
    xa8_b, xb16_b = {}, {}
    for b in range(B):
        a8 = np.empty((NL, P, 3, ND, SCL), dtype=ml_dtypes.float8_e4m3)
        b16 = np.empty((P, 3, ND, SC), dtype=np.float16)
        for i, x in enumerate((q, k, v)):
            # x[b]: [S, D] -> chunks [NL, SCL, ND, P] -> [NL, P, ND, SCL]
            x8 = _e4m3(np.asarray(x[b], np.float32))
            a8[:, :, i] = x8.reshape(NL, SCL, ND, P).transpose(0, 3, 2, 1)
            b16[:, i] = (
                np.asarray(x[b][:SC], np.float32)
                .astype(np.float16)
                .reshape(SC, ND, P)
                .transpose(2, 1, 0)
            )
        xa8_b[b] = a8.reshape(NL, P, 3 * ND * SCL)
        xb16_b[b] = b16.reshape(P, 3 * ND * SC)
    in_maps = []
    for c in range(NCORES):
        b, h = c // 2, c % 2
        e0 = h * EH
        w16 = np.empty((P, 3, ND, EH), dtype=np.float16)
        w8 = np.empty((P, 3, ND, EH), dtype=ml_dtypes.float8_e4m3)
        for i, W in enumerate((Wq, Wk, Wv)):
            # W^T[:, e0:e0+EH]: [D, EH] -> [ND, P, EH] -> [P, ND, EH]
            wt = np.ascontiguousarray(np.asarray(W, np.float32).T[:, e0 : e0 + EH])
            w16[:, i] = wt.astype(np.float16).reshape(ND, P, EH).transpose(1, 0, 2)
            w8[:, i] = _e4m3(wt * WS).reshape(ND, P, EH).transpose(1, 0, 2)
        in_maps.append(
            {
                "xa8": xa8_b[b],
                "xb16": xb16_b[b],
                "wb16": w16.reshape(P, 3 * ND * EH),
                "wb8": w8.reshape(P, 3 * ND * EH),
            }
        )
    return in_maps


_NC_CACHE = None


def _get_nc():
    global _NC_CACHE
    if _NC_CACHE is None:
        _NC_CACHE = build_nc()
    return _NC_CACHE


def run_spmd(v, k, q, Wq, Wk, Wv, **kwargs):
    """Run on 8 cores; returns (assembled output [B,S,E] fp32, raw results)."""
    nc = _get_nc()
    in_maps = _host_prep(v, k, q, Wq, Wk, Wv)
    res = run_bass_kernel_spmd(nc, in_maps, core_ids=list(range(NCORES)), **kwargs)
    full = np.empty((B, S, E), dtype=np.float32)
    for c in range(NCORES):
        b, h = c // 2, c % 2
        full[b, :, h * EH : (h + 1) * EH] = res.results[c]["out"].T.astype(np.float32)
    return full, res


def kernel(v, k, q, Wq, Wk, Wv):
    v, k, q, Wq, Wk, Wv = (
        np.asarray(a, dtype=np.float32) for a in (v, k, q, Wq, Wk, Wv)
    )
    full, _ = run_spmd(v, k, q, Wq, Wk, Wv)
    return full


# revision 65
# speedup vs baseline: 18.8734x; 1.3334x over previous
"""Trainium2 Bass kernel for nn_AttnFreeLayer (linear-attention-style layer).

Computes, for inputs q,k,v [B,S,D] and weights Wq,Wk,Wv [E,D] (E=D):
    q_in = elu(q @ Wq^T) + 1
    k_in = elu(k @ Wk^T) + 1
    v_in = v @ Wv^T
    kv_in = k_in * v_in
    out = q_in * (kv_in + cumsum_s(kv_in)) / cumsum_s(k_in)

Sharding: 8 cores = 4 batches x 2 halves of the output dim E; no
collectives. Each core computes out[b, e0:e0+512, :] in a TRANSPOSED
[e, s] layout: the projection matmuls put W chunks stationary and x^T
moving, so outputs land with e on partitions and s on the free dim.
The seq-cumsum runs along the free dimension via the DVE
tensor_tensor_scan instruction (chained across s-chunks through its
`initial` operand).

v8 (223us/pass measured, repeat-delta min-of-12, vs 574us harness
baseline = 2.57x; rel err 1.5118e-3 vs the 2e-2 gate). Steps measured
along the way: v3 recip-fix 337us, v4 +NUM_CUMSUM fusion 296us,
v5 +custom 1x k-scan (per-ec full tiles) 264us, v6 +CK_DEN_ACC full
den fusion 258us, v7 +minexp elu (ACT had become critical) 239us,
v8 +NUM_KV (kv-mul folded into the cumsum op) 223us. Changes vs the
original baseline:
- DMA coalescing: q/k/v packed into ONE DRAM tensor laid out so each
  load is a single 24KB-per-partition contiguous transfer (8 x-loads
  for the whole pass instead of 45), and output is staged in SBUF and
  stored 2048 columns at a time (16 stores instead of 64). 27 DMAs
  total vs 118.
- Custom scan-in-body DVE ops (registered below at import), replacing
  the native 2cyc/elem scans and the ~6cyc/elem InstReciprocal
  (probe-measured; it was ~192us of the original pass!) with 1x fused
  passes:
  * ANT_NUM_KV_CUMSUM: num = k1*v1 + carry + cumsum(k1*v1) — the kv
    multiply never materializes; carry = num[-1] - k1[-1]*v1[-1]
    via two [P,1] ops.
  * ANT_CK_DEN_ACC: den = 1NR-approx-recip(carry + cumsum(k1)) with
    a BITWISE_NOT exponent-flip seed (max rel err 1.7e-3), fp32
    MIN-accum = den[last] so the next carry is one exact [P,1]
    reciprocal. ck is never materialized. accum_init=C0 reuses the
    carry as the MIN seed (carry > 1 > every den, so it never wins).
  * Custom-op outputs must be FULL tiles — a free-dim-offset output
    slice crashes the runtime; per-ec tiles fixed that.
  * tensor_tensor_reduce crashes this runtime outright (minimal
    1-core repro: NRT_EXEC_UNIT_UNRECOVERABLE) — carries avoid it.
- elu(x)+1 = min(exp(x), relu(x)+1) (e^x >= 1+x everywhere): 2 ACT
  table passes + a 4x tensor_scalar(+1) + a 2x min. With the den/num
  work fused away, ACT became the critical engine, flipping the
  earlier act3-vs-minexp tradeoff (both modes kept as flags).

Precision: hybrid fp16/fp8 as before. First 512 seq positions use
fp16 matmuls, the remaining 7680 use fp8-e4m3 DoubleRow matmuls.
fp8 weights pre-scaled by 16 (avoids e4m3 subnormals); downstream ACT
ops undo it via their free `scale`.
"""

import sys

for _p in ("/opt/trn_rl_repo",):
    if _p not in sys.path:
        sys.path.insert(0, _p)

from contextlib import ExitStack

import numpy as np
import ml_dtypes

import concourse.bass as bass
import concourse.tile as tile
from concourse import bacc
from concourse import mybir
from concourse.alu_op_type import AluOpType
from concourse.bass_utils import run_bass_kernel_spmd

FP8 = mybir.dt.float8e4
FP16 = mybir.dt.float16
FP32 = mybir.dt.float32
AF = mybir.ActivationFunctionType
DR = mybir.MatmulPerfMode.DoubleRow

# --- custom fused DVE op: t1 = q1 * (kv + carry + cumsum(kv)) --------------
# Registered via the documented extension point (append to dve_ops.OPS);
# the per-NEFF DVE table is generated from the registry by name, and the
# uops sha is computed here the same way DveOp.compile() checks it.
from concourse import dve_ops as _dops
from concourse import dve_spec as _dspec
from concourse.dve_uop import DveOpSpec as _DveOpSpec


def _ref_t1_fused(in0, in1, s0, s1, imm2):
    return (
        in0.astype(np.float32)
        * (in1 + s0 + np.cumsum(in1.astype(np.float32), axis=-1))
    ).astype(np.float32)


def _register_t1_fused():
    name = "ANT_T1_CUMSUM_FUSED"
    for o in _dops.OPS:
        if o.name == name:
            return o
    body = _dspec.Src0 * (
        _dspec.Src1 + _dspec.scan(_dspec.AluOp.ADD, _dspec.Src1, init=_dspec.C0)
    )
    spec = _dspec.Spec(body=body, reference=_ref_t1_fused)
    row = _dops._CUSTOM_DVE_ROW_BASE + len(_dops.OPS)
    shas = {}
    for ver in ("v3", "v4"):
        uops = _dspec.lower(spec, ver=ver)
        shas[ver] = _DveOpSpec(
            name=name, opcode=row, uops=uops, rd1_en=_dspec._has_src1(spec)
        ).sha(ver)
    op = _dops.DveOp(name, spec, subdim=False, uops_sha=shas)
    _dops.OPS.append(op)
    _dops.CUSTOM_DVE_SPECS[name] = spec
    _dops._SUB_OPCODE_FOR_NAME[name] = row
    return op


T1_FUSED = _register_t1_fused()


def _register_dve_op(name, spec):
    for o in _dops.OPS:
        if o.name == name:
            return o
    row = _dops._CUSTOM_DVE_ROW_BASE + len(_dops.OPS)
    shas = {}
    for ver in ("v3", "v4"):
        uops = _dspec.lower(spec, ver=ver)
        shas[ver] = _DveOpSpec(
            name=name, opcode=row, uops=uops, rd1_en=_dspec._has_src1(spec)
        ).sha(ver)
    op = _dops.DveOp(name, spec, subdim=False, uops_sha=shas)
    _dops.OPS.append(op)
    _dops.CUSTOM_DVE_SPECS[name] = spec
    _dops._SUB_OPCODE_FOR_NAME[name] = row
    return op


def _ref_t1_fused_b(in0, in1, s0, s1, imm2):
    return (
        in0.astype(np.float32)
        * (in1 + s0 + np.cumsum(in1.astype(np.float32), axis=-1))
    ).astype(np.float32)


# variant B: carry rides as a body-side constant add (scan init stays the
# ADD identity) — same value as T1_FUSED, different uop schedule
T1_FUSED_B = _register_dve_op(
    "ANT_T1_CUMSUM_FUSED_B",
    _dspec.Spec(
        body=_dspec.Src0
        * (
            _dspec.Src1
            + _dspec.C0
            + _dspec.scan(_dspec.AluOp.ADD, _dspec.Src1, init=_dspec.Zero)
        ),
        reference=_ref_t1_fused_b,
    ),
)


def _ref_ck_den(in0, in1, s0, s1, imm2):
    ck = s0 + np.cumsum(in0.astype(np.float32), axis=-1)
    not_x = (~ck.astype(np.float32).view(np.int32)).view(np.float32)
    y0 = not_x * np.float32(s1)
    return (y0 * (np.float32(imm2) - ck * y0)).astype(np.float32)


def _make_ck_den_spec():
    ck = _dspec.C0 + _dspec.scan(_dspec.AluOp.ADD, _dspec.Src0, init=_dspec.Zero)
    nx = _dspec.Bin(_dspec.AluOp.BITWISE_NOT, ck, ck)
    y0 = nx * _dspec.C1
    return _dspec.Spec(body=y0 * (_dspec.C2 - ck * y0), reference=_ref_ck_den)


# den = approx-1/(carry + cumsum(k1)): BITWISE_NOT seed + ONE inline NR
# pass (max rel err 1.73e-3 on [1e-3, 3e4] with the stock Chebyshev pair;
# checked numerically) — replaces the native k-scan AND the reciprocal
CK_DEN_CONSTS = {"s1": -0.23549792, "imm2": 2.0017324}
CK_DEN = _register_dve_op("ANT_CK_DEN_FUSED", _make_ck_den_spec())


def _ref_ck_den_acc(in0, in1, s0, s1, imm2):
    body = _ref_ck_den(in0, in1, s0, s1, imm2)
    return body, np.minimum(body.min(axis=-1, keepdims=True), s0)


def _make_ck_den_acc_spec():
    ck = _dspec.C0 + _dspec.scan(_dspec.AluOp.ADD, _dspec.Src0, init=_dspec.Zero)
    nx = _dspec.Bin(_dspec.AluOp.BITWISE_NOT, ck, ck)
    y0 = nx * _dspec.C1
    # accum seed C0 = the incoming carry ck_start: for chunks with
    # ck_start > 1 every den < 1/ck_start < ck_start, so the seed never
    # wins the MIN; chunk 0 (C0 = 0) ignores accum_out and recovers its
    # carry from den[:, -1:] instead
    return _dspec.Spec(
        body=y0 * (_dspec.C2 - ck * y0),
        accum=_dspec.AluOp.MIN,
        accum_init=_dspec.C0,
        reference=_ref_ck_den_acc,
    )


# CK_DEN + accum_out = min(den) = den[last] (den is positive decreasing),
# kept in fp32 so the next chunk's carry 1/min(den) skips the fp16 round
CK_DEN_ACC = _register_dve_op("ANT_CK_DEN_ACC", _make_ck_den_acc_spec())


def _ref_num_cumsum(in0, in1, s0, s1, imm2):
    return (in0 + s0 + np.cumsum(in0.astype(np.float32), axis=-1)).astype(np.float32)


# num = kv + carry + cumsum(kv): ONE 1x DVE pass replacing the 2cyc/elem
# native kv-scan plus the num tensor_add (the carry chain is recovered as
# num[-1] - kv[-1], a [P,1] subtract)
NUM_CUMSUM = _register_dve_op(
    "ANT_NUM_CUMSUM",
    _dspec.Spec(
        body=_dspec.Src0
        + _dspec.C0
        + _dspec.scan(_dspec.AluOp.ADD, _dspec.Src0, init=_dspec.Zero),
        reference=_ref_num_cumsum,
    ),
)


def _ref_ck_cumsum(in0, in1, s0, s1, imm2):
    return (s0 + np.cumsum(in0.astype(np.float32), axis=-1)).astype(np.float32)


def _ref_num_kv(in0, in1, s0, s1, imm2):
    kv = in0.astype(np.float32) * in1.astype(np.float32)
    return (kv + s0 + np.cumsum(kv, axis=-1)).astype(np.float32)


def _make_num_kv_spec():
    kv = _dspec.Src0 * _dspec.Src1  # shared subexpression: computed once
    return _dspec.Spec(
        body=kv + _dspec.C0 + _dspec.scan(_dspec.AluOp.ADD, kv, init=_dspec.Zero),
        reference=_ref_num_kv,
    )


# num = k1*v1 + carry + cumsum(k1*v1): folds the kv multiply into the
# fused cumsum pass, so kv is never materialized (the carry needs only
# kv[-1] = k1[-1]*v1[-1], two [P,1] ops)
NUM_KV = _register_dve_op("ANT_NUM_KV_CUMSUM", _make_num_kv_spec())


# ck = carry + cumsum(k1): custom 1x pass replacing the 2cyc/elem native
# scan (carry still read from the materialized fp32 ckg's last column).
# DISABLED (ck_custom=False): crashes at runtime when the output AP is a
# free-dim-offset slice of a wider tile (NUM_CUMSUM with a full-tile out
# works); needs a dedicated per-ec tile to retry.
CK_CUMSUM = _register_dve_op(
    "ANT_CK_CUMSUM",
    _dspec.Spec(
        body=_dspec.C0 + _dspec.scan(_dspec.AluOp.ADD, _dspec.Src0, init=_dspec.Zero),
        reference=_ref_ck_cumsum,
    ),
)

B, S, D, E = 4, 8192, 1024, 1024
NCORES = 8
EH = E // 2  # e-half per core
P = 128  # partition block
SC = 512  # s-chunk width (PSUM bank = 512 fp32)
N_SC = S // SC  # 16
N_EC = EH // P  # 4
ND = D // P  # 8 contraction chunks
WS = 16.0  # fp8 weight prescale
SCL = 1024  # x-load width (2 s-chunks per DMA)
NL = S // SCL  # 8 loads
SCO = 2048  # out-store width (4 s-chunks per DMA)


def build_nc(
    repeat=1,
    debug=False,
    den_mode="ckden",
    elu_mode="minexp",
    fuse_t1=True,
    fuse_kv=True,
    ck_custom=True,
    gps_min=False,
    scan_eng="vector",
    gps=False,
    probe="none",
):
    """den_mode: "recipfast" (RECIPROCAL_APPROX_FAST custom DVE op, 1
    cyc/elem vs ~6 for InstReciprocal on HW; needs fp32 ck), "recip"
    (InstReciprocal), or "lnexp" (exp(-ln(ck)) on ACT).
    elu_mode: "act3" = exp(-relu(-x))+relu(x) via 3 ACT passes + one 2x
    DVE add (ACT has headroom; DVE is critical). "minexp" =
    min(exp(x), relu(x)+1): one fewer ACT pass but +1 DVE op.
    fuse_t1: use the ANT_T1_CUMSUM_FUSED custom DVE op
    (t1 = q1*(kv + carry + cumsum(kv)) in ONE 1x pass, replacing
    scan_kv + num-add + t1-mul), with the kv chunk-sum carried by
    tensor_tensor_reduce's free accumulator.
    scan_eng: must be "vector" — TensorTensorScanArith is NOT a valid
    opcode on the Pool engine (walrus codegen asserts).
    probe: timing-only variants with WRONG numerics — "noscan",
    "norecip", "noelu", "nomm"."""
    nc = bacc.Bacc("TRN2", target_bir_lowering=False, debug=debug)

    # packed inputs: per-partition contiguous 24KB loads
    xa8 = nc.declare_dram_parameter("xa8", [NL, P, 3 * ND * SCL], FP8, isOutput=False)
    xb16 = nc.declare_dram_parameter("xb16", [P, 3 * ND * SC], FP16, isOutput=False)
    wb16 = nc.declare_dram_parameter("wb16", [P, 3 * ND * EH], FP16, isOutput=False)
    wb8 = nc.declare_dram_parameter("wb8", [P, 3 * ND * EH], FP8, isOutput=False)
    outp = nc.declare_dram_parameter("out", [EH, S], FP16, isOutput=True)

    with tile.TileContext(nc) as tc, ExitStack() as ctx:
        wpool = ctx.enter_context(tc.tile_pool(name="w", bufs=1))
        x8pool = ctx.enter_context(tc.tile_pool(name="x8", bufs=2))
        apool = ctx.enter_context(tc.tile_pool(name="act", bufs=2))
        vpool = ctx.enter_context(tc.tile_pool(name="vv", bufs=2))
        kvpool = ctx.enter_context(tc.tile_pool(name="kv", bufs=3))
        cpool = ctx.enter_context(tc.tile_pool(name="cum", bufs=2))
        dpool = ctx.enter_context(tc.tile_pool(name="den", bufs=2))
        opool = ctx.enter_context(tc.tile_pool(name="out", bufs=2))
        pp = ctx.enter_context(tc.tile_pool(name="pqk", bufs=3, space="PSUM"))
        pvp = ctx.enter_context(tc.tile_pool(name="pv", bufs=2, space="PSUM"))

        # --- resident weights + first-chunk fp16 x ---
        w16_t = wpool.tile([P, 3, ND, EH], FP16, tag="w16")
        nc.sync.dma_start(
            out=w16_t[:], in_=wb16[:].rearrange("p (i j e) -> p i j e", i=3, j=ND)
        )
        w8_t = wpool.tile([P, 3, ND, EH], FP8, tag="w8")
        nc.sync.dma_start(
            out=w8_t[:], in_=wb8[:].rearrange("p (i j e) -> p i j e", i=3, j=ND)
        )
        x16_t = wpool.tile([P, 3, ND, SC], FP16, tag="x16")
        nc.sync.dma_start(
            out=x16_t[:], in_=xb16[:].rearrange("p (i j s) -> p i j s", i=3, j=ND)
        )

        ck_dt = FP32 if den_mode == "recipfast" else FP16

        def main_body():
            carry_k = [[None] * N_EC]  # per-ec [P, SC] ck tile of prev sc
            carry_kv = [None]  # fused: [P, N_EC] fp32 chunk-sum carries
            otiles = [None] * N_EC

            def emit_unit(sc, ec, x8t, ckvg, kvc):
                # fused mode: kvc = [P, N_EC] fp32 chunk-sum carry tile;
                # non-fused: ckvg = [P, N_EC*SC] cumsum group tile
                fp8 = sc > 0
                e0 = ec * P
                sscale = (1.0 / WS) if fp8 else 1.0
                half = (sc % 2) if fp8 else 0
                pqk = pp.tile([P, 2 * SC], FP32, tag="pqk")
                pv_ = pvp.tile([P, SC], FP32, tag="pv")
                for i in range(0 if probe != "nomm" else 3, 3):
                    dst = pqk[:, i * SC : (i + 1) * SC] if i < 2 else pv_[:]
                    if fp8:
                        for j in range(ND // 2):
                            nc.tensor.matmul(
                                dst,
                                lhsT=w8_t[:, i, 2 * j : 2 * j + 2, e0 : e0 + P],
                                rhs=x8t[
                                    :,
                                    i,
                                    2 * j : 2 * j + 2,
                                    half * SC : (half + 1) * SC,
                                ],
                                start=(j == 0),
                                stop=(j == ND // 2 - 1),
                                perf_mode=DR,
                                skip_group_check=True,
                            )
                    else:
                        for j in range(ND):
                            nc.tensor.matmul(
                                dst,
                                lhsT=w16_t[:, i, j, e0 : e0 + P],
                                rhs=x16_t[:, i, j, :],
                                start=(j == 0),
                                stop=(j == ND - 1),
                                skip_group_check=True,
                            )
                # elu(x)+1
                v1 = vpool.tile([P, SC], FP16, tag="v1")
                nc.scalar.activation(v1[:], pv_[:], AF.Copy, scale=sscale)
                qk1 = apool.tile([P, 2 * SC], FP16, tag="qk1")
                if probe == "noelu":
                    nc.scalar.activation(qk1[:], pqk[:], AF.Copy, scale=sscale)
                elif elu_mode == "act3":
                    # exp(-relu(-x)) + relu(x): 3 ACT table passes + one
                    # 2x-rate DVE add — minimal DVE load
                    rn = apool.tile([P, 2 * SC], FP16, tag="rn")
                    nc.scalar.activation(rn[:], pqk[:], AF.Relu, scale=-sscale)
                    ex = apool.tile([P, 2 * SC], FP16, tag="ex")
                    nc.scalar.activation(ex[:], rn[:], AF.Exp, scale=-1.0)
                    rp = apool.tile([P, 2 * SC], FP16, tag="rp")
                    nc.scalar.activation(rp[:], pqk[:], AF.Relu, scale=sscale)
                    nc.vector.tensor_add(qk1[:], ex[:], rp[:])
                else:  # minexp
                    ex = apool.tile([P, 2 * SC], FP16, tag="ex")
                    nc.scalar.activation(ex[:], pqk[:], AF.Exp, scale=sscale)
                    rp = apool.tile([P, 2 * SC], FP16, tag="rp")
                    nc.scalar.activation(rp[:], pqk[:], AF.Relu, scale=sscale)
                    rp1 = apool.tile([P, 2 * SC], FP16, tag="rp1")
                    nc.vector.tensor_scalar_add(rp1[:], rp[:], 1.0)
                    (nc.gpsimd if gps_min else nc.vector).tensor_tensor(
                        qk1[:], ex[:], rp1[:], AluOpType.min
                    )
                q1 = qk1[:, 0:SC]
                k1 = qk1[:, SC : 2 * SC]
                cs = slice(ec * SC, (ec + 1) * SC)
                seng = nc.gpsimd if scan_eng == "gpsimd" else nc.vector
                ckt = None
                if den_mode == "ckden":
                    # den = 1NR-recip(carry + cumsum(k1)) in ONE DVE pass;
                    # fp32 min-accum = den[last] -> next carry via a [P,1]
                    # exact reciprocal (no materialized ck at all)
                    ik = 0.0 if sc == 0 else carry_k[0][ec][:, 0:1]
                    dent = dpool.tile([P, SC], FP16, tag=f"den{ec}", name=f"den{ec}")
                    dacc = dpool.tile([P, 1], FP32, tag=f"da{ec}", name=f"da{ec}")
                    cc = CK_DEN_CONSTS
                    nc.vector._custom_dve(
                        CK_DEN_ACC,
                        out=dent[:],
                        in0=k1,
                        s0=ik,
                        s1=cc["s1"],
                        imm2=cc["imm2"],
                        accum_out=dacc[:],
                    )
                    ckc = dpool.tile([P, 1], FP32, tag=f"ckc{ec}", name=f"ckc{ec}")
                    nc.vector.reciprocal(
                        ckc[:], dacc[:] if sc > 0 else dent[:, SC - 1 : SC]
                    )
                    carry_k[0][ec] = ckc
                else:
                    # per-ec FULL ck tile: custom ops crash on free-dim-offset
                    # output slices, and per-ec den drops the 4-scan barrier
                    ckt = cpool.tile([P, SC], ck_dt, tag=f"ck{ec}", name=f"ck{ec}")
                    ik = 0.0 if sc == 0 else carry_k[0][ec][:, SC - 1 : SC]
                    if probe == "noscan":
                        nc.vector.tensor_copy(ckt[:], k1)
                    elif ck_custom:
                        nc.vector._custom_dve(CK_CUMSUM, out=ckt[:], in0=k1, s0=ik)
                    else:
                        seng.tensor_tensor_scan(
                            ckt[:], k1, k1, ik, op0=AluOpType.add, op1=AluOpType.bypass
                        )
                    carry_k[0][ec] = ckt
                kv = None
                t1 = kvpool.tile([P, SC], FP16, tag="t1")
                if fuse_t1 and fuse_kv:
                    ikv = 0.0 if sc == 0 else carry_kv[0][:, ec : ec + 1]
                    num = kvpool.tile([P, SC], FP16, tag="num")
                    nc.vector._custom_dve(
                        NUM_KV, out=num[:], in0=k1, in1=v1[:], s0=ikv
                    )
                    # carry = num[-1] - kv[-1]; kv[-1] from two [P,1] ops
                    kvl = kvpool.tile([P, 1], FP32, tag="kvl", name="kvl")
                    nc.vector.tensor_tensor(
                        kvl[:],
                        qk1[:, 2 * SC - 1 : 2 * SC],
                        v1[:, SC - 1 : SC],
                        AluOpType.mult,
                    )
                    nc.vector.tensor_tensor(
                        kvc[:, ec : ec + 1],
                        num[:, SC - 1 : SC],
                        kvl[:],
                        AluOpType.subtract,
                    )
                    nc.vector.tensor_mul(t1[:], q1, num[:])
                elif fuse_t1:
                    kv = kvpool.tile([P, SC], FP16, tag="kv")
                    nc.vector.tensor_mul(kv[:], k1, v1[:])
                    ikv = 0.0 if sc == 0 else carry_kv[0][:, ec : ec + 1]
                    num = kvpool.tile([P, SC], FP16, tag="num")
                    nc.vector._custom_dve(NUM_CUMSUM, out=num[:], in0=kv[:], s0=ikv)
                    # inclusive ckv carry for the next chunk
                    nc.vector.tensor_tensor(
                        kvc[:, ec : ec + 1],
                        num[:, SC - 1 : SC],
                        kv[:, SC - 1 : SC],
                        AluOpType.subtract,
                    )
                    nc.vector.tensor_mul(t1[:], q1, num[:])
                else:
                    kv = kvpool.tile([P, SC], FP16, tag="kv")
                    (nc.gpsimd if gps else nc.vector).tensor_mul(kv[:], k1, v1[:])
                    ikv = (
                        0.0
                        if sc == 0
                        else carry_kv[0][:, (ec + 1) * SC - 1 : (ec + 1) * SC]
                    )
                    if probe == "noscan":
                        nc.vector.tensor_copy(ckvg[:, cs], kv[:])
                    else:
                        seng.tensor_tensor_scan(
                            ckvg[:, cs],
                            kv[:],
                            kv[:],
                            ikv,
                            op0=AluOpType.add,
                            op1=AluOpType.bypass,
                        )
                    num = kvpool.tile([P, SC], FP16, tag="num")
                    (nc.gpsimd if gps else nc.vector).tensor_add(
                        num[:], ckvg[:, cs], kv[:]
                    )
                    nc.vector.tensor_mul(t1[:], q1, num[:])
                # 1/ck for THIS ec block (no cross-ec barrier), then the
                # output column block
                if den_mode == "ckden":
                    pass  # dent already computed above
                else:
                    dent = dpool.tile([P, SC], FP16, tag=f"den{ec}", name=f"den{ec}")
                if den_mode == "ckden":
                    pass
                elif probe == "norecip":
                    nc.vector.tensor_copy(dent[:], ckt[:])
                elif den_mode == "recipfast":
                    c = _dops.RECIP_APPROX_FAST_CONSTS
                    nc.vector._custom_dve(
                        _dops.RECIPROCAL_APPROX_FAST,
                        out=dent[:],
                        in0=ckt[:],
                        s0=c["s0"],
                        s1=c["s1"],
                        imm2=c["imm2"],
                    )
                elif den_mode == "lnexp":
                    lnk = dpool.tile([P, SC], FP16, tag=f"lnk{ec}", name=f"lnk{ec}")
                    nc.scalar.activation(lnk[:], ckt[:], AF.Ln)
                    nc.scalar.activation(dent[:], lnk[:], AF.Exp, scale=-1.0)
                else:
                    with nc.allow_low_precision(reason="1/k_prefix; 5e-4 rel ok"):
                        nc.vector.reciprocal(dent[:], ckt[:])
                g = sc % (SCO // SC)
                nc.vector.tensor_mul(
                    otiles[ec][:, g * SC : (g + 1) * SC], t1[:], dent[:]
                )

            x8t = None
            for sc in range(N_SC):
                # fp8 chunk sc lives in load l=sc//2, half sc%2; a new load
                # is needed at sc=1 (l=0) and every even sc>=2 (l advances)
                if sc == 1 or (sc >= 2 and sc % 2 == 0):
                    x8t = x8pool.tile([P, 3, ND, SCL], FP8, tag="x8")
                    nc.sync.dma_start(
                        out=x8t[:],
                        in_=xa8[sc // 2, :, :].rearrange(
                            "p (i j s) -> p i j s", i=3, j=ND
                        ),
                    )
                g = sc % (SCO // SC)  # position within the out-store group
                if g == 0:
                    for ec in range(N_EC):
                        otiles[ec] = opool.tile(
                            [P, SCO], FP16, tag=f"ot{ec}", name=f"ot{ec}"
                        )
                ckvg = (
                    None
                    if fuse_t1
                    else cpool.tile([P, N_EC * SC], FP16, tag="ckvg", name="ckvg")
                )
                kvc = (
                    cpool.tile([P, N_EC], FP32, tag="kvc", bufs=3, name="kvc")
                    if fuse_t1
                    else None
                )
                for ec in range(N_EC):
                    emit_unit(sc, ec, x8t, ckvg, kvc)
                if g == SCO // SC - 1:
                    for ec in range(N_EC):
                        nc.sync.dma_start(
                            out=outp[
                                ec * P : (ec + 1) * P,
                                (sc - g) * SC : (sc + 1) * SC,
                            ],
                            in_=otiles[ec][:],
                        )
                carry_kv[0] = kvc if fuse_t1 else ckvg

        if repeat == 1:
            main_body()
        else:
            with tc.For_i(0, repeat, 1):
                main_body()

    nc.compile()
    return nc


def _e4m3(x):
    return np.clip(x, -240, 240).astype(ml_dtypes.float8_e4m3)


def _host_prep(v, k, q, Wq, Wk, Wv):
    """Build the 8 per-core input maps (packed, per-partition contiguous)."""
    xa8_b, xb16_b = {}, {}
    for b in range(B):
        a8 = np.empty((NL, P, 3, ND, SCL), dtype=ml_dtypes.float8_e4m3)
        b16 = np.empty((P, 3, ND, SC), dtype=np.float16)
        for i, x in enumerate((q, k, v)):
            # x[b]: [S, D] -> chunks [NL, SCL, ND, P] -> [NL, P, ND, SCL]
            x8 = _e4m3(np.asarray(x[b], np.float32))
            a8[:, :, i] = x8.reshape(NL, SCL, ND, P).transpose(0, 3, 2, 1)
            b16[:, i] = (
                np.asarray(x[b][:SC], np.float32)
                .astype(np.float16)
                .reshape(SC, ND, P)
                .transpose(2, 1, 0)
            )
        xa8_b[b] = a8.reshape(NL, P, 3 * ND * SCL)
        xb16_b[b] = b16.reshape(P, 3 * ND * SC)
    in_maps = []
    for c in range(NCORES):
        b, h = c // 2, c % 2
        e0 = h * EH
        w16 = np.empty((P, 3, ND, EH), dtype=np.float16)
        w8 = np.empty((P, 3, ND, EH), dtype=ml_dtypes.float8_e4m3)
        for i, W in enumerate((Wq, Wk, Wv)):
            # W^T[:, e0:e0+EH]: [D, EH] -> [ND, P, EH] -> [P, ND, EH]
            wt = np.ascontiguousarray(np.asarray(W, np.float32).T[:, e0 : e0 + EH])
            w16[:, i] = wt.astype(np.float16).reshape(ND, P, EH).transpose(1, 0, 2)
            w8[:, i] = _e4m3(wt * WS).reshape(ND, P, EH).transpose(1, 0, 2)
        in_maps.append(
            {
                "xa8": xa8_b[b],
                "xb16": xb16_b[b],
                "wb16": w16.reshape(P, 3 * ND * EH),
                "wb8": w8.reshape(P, 3 * ND * EH),
            }
        )
    return in_maps


_NC_CACHE = None


def _get_nc():
    global _NC_CACHE
    if _NC_CACHE is None:
        _NC_CACHE = build_nc()
    return _NC_CACHE


def run_spmd(v, k, q, Wq, Wk, Wv, **kwargs):
    """Run on 8 cores; returns (assembled output [B,S,E] fp32, raw results)."""
    nc = _get_nc()
    in_maps = _host_prep(v, k, q, Wq, Wk, Wv)
    res = run_bass_kernel_spmd(nc, in_maps, core_ids=list(range(NCORES)), **kwargs)
    full = np.empty((B, S, E), dtype=np.float32)
    for c in range(NCORES):
        b, h = c // 2, c % 2
        full[b, :, h * EH : (h + 1) * EH] = res.results[c]["out"].T.astype(np.float32)
    return full, res


def kernel(v, k, q, Wq, Wk, Wv):
    v, k, q, Wq, Wk, Wv = (
        np.asarray(a, dtype=np.float32) for a in (v, k, q, Wq, Wk, Wv)
    )
    full, _ = run_spmd(v, k, q, Wq, Wk, Wv)
    return full
